# revision 1
# baseline (speedup 1.0000x reference)
"""AttLoRA MoE-routing kernel for 8 Trainium2 NeuronCores.

Reference computation (per problem nn_AttLoRAModule_85839216378078):
    base  = x @ W_org.T                                    [B,S,OUT]
    q     = x.mean(axis=1) @ Wq.T                          [B,K]
    coef  = softmax(q @ lora_keys.T / sqrt(K))             [B,E]
    h     = x @ lora_down[e]                               [B,S,E,R]
    delta = sum_e coef[b,e] * (h[...,e,:] @ lora_up[e])    [B,S,OUT]
    out   = base + delta * SCALE

Sharding: 8 cores = 4 batches x 2 OUT-halves.  Core c handles batch c//2,
output columns [(c%2)*2048, (c%2+1)*2048).  Each core sees the full x[b], so
the router (softmax coefficients) is computed on-device per core with no
collectives.

Device strategy (per core):
  - All matmuls in float32r (full PE rate at N>=512, ~1e-4 rel err).
  - K(=IN)-split into 2 passes of 2048 contraction rows so the x.T slab
    [2048, 2048] f32 (16 MiB) stays SBUF-resident per pass; W / lora_down
    stream through a shared pool; output accumulated in DRAM via a second
    pass with accum_op=add DMA.
  - LoRA path: tT[er,s] = (x @ lora_down).T accumulated across passes into a
    bf16 tile; router coeffs folded into lora_up tiles (bf16); 4 delta
    matmuls appended to each pass-2 PSUM accumulation group.
  - Router: proj = x @ ((Wq.T @ keys.T)/(S*sqrt(K))) accumulated in one PSUM
    bank across both passes, reduced over S, softmax on one partition, then
    broadcast to partitions via rank-1 outer-product matmuls.
"""

import math
import os

import numpy as np

import concourse.bacc as bacc
import concourse.mybir as mybir
import concourse.tile as tile
from concourse.bass_utils import run_bass_kernel_spmd

# Problem shapes (hardcoded per contest contract)
B, S, IN, OUT = 4, 2048, 4096, 4096
E, R, K = 8, 64, 128
ER = E * R            # 512
OH = OUT // 2         # 2048 output cols per core
P = 128
NP = 2                # contraction passes
IOP = IN // NP // P   # 16 io-subtiles per pass
SCALE = 1.0           # (alpha/lora_dim) * multiplier

F32 = mybir.dt.float32
F32R = mybir.dt.float32r
BF16 = mybir.dt.bfloat16

_NC_CACHE = {}


def _build_nc():
    nc = bacc.Bacc("TRN2", target_bir_lowering=False, debug=False)

    # f32r inputs: same 4-byte fp32 payload, but typed float32r end-to-end so
    # the BIR verifier accepts them as FP32r-matmul operands.
    xT = nc.dram_tensor("xT", [IN, S], F32R, kind="ExternalInput")
    wT = nc.dram_tensor("wT", [IN, OH], F32R, kind="ExternalInput")
    ldn = nc.dram_tensor("ldn", [IN, ER], F32R, kind="ExternalInput")
    lup = nc.dram_tensor("lup", [ER, OH], F32, kind="ExternalInput")
    mk = nc.dram_tensor("mk", [IN, E], F32R, kind="ExternalInput")
    cind = nc.dram_tensor("cind", [E, ER], F32, kind="ExternalInput")
    out = nc.dram_tensor("out", [S, OH], F32, kind="ExternalOutput")
    # pass-1 partial sums; read back and added during pass-2 eviction (avoids
    # SWDGE read-modify-write accumulate DMAs, which serialize on hardware)
    out_p1 = nc.dram_tensor("out_p1", [S, OH], F32)

    xT_ap, wT_ap, ldn_ap, lup_ap, mk_ap, cind_ap, out_ap, out_p1_ap = (
        t.ap() for t in (xT, wT, ldn, lup, mk, cind, out, out_p1)
    )

    trace_sim = os.environ.get("KERNEL_SIM_TRACE", "0") == "1"
    with tile.TileContext(nc, trace_sim=trace_sim) as tc:
        with (
            tc.tile_pool(name="xpool", bufs=1) as xpool,
            tc.tile_pool(name="spool", bufs=3) as spool,
            tc.tile_pool(name="tpool", bufs=1) as tpool,
            tc.tile_pool(name="lpool", bufs=2) as lpool,
            tc.tile_pool(name="opool", bufs=2) as opool,
            tc.tile_pool(name="rpool", bufs=1) as rpool,
            tc.tile_pool(name="ptp", bufs=2, space="PSUM") as ptp,
            tc.tile_pool(name="pop", bufs=4, space="PSUM") as pop,
            tc.tile_pool(name="prp", bufs=1, space="PSUM") as prp,
            tc.tile_pool(name="pccp", bufs=1, space="PSUM") as pccp,
        ):
            # --- persistent tiles ---
            mk_sb = rpool.tile([P, IN // P, E], F32R, name="mk_sb")
            nc.sync.dma_start(mk_sb[:], mk_ap.rearrange("(io pp) e -> pp io e", pp=P))
            cind_sb = rpool.tile([E, ER], F32, name="cind_sb")
            nc.sync.dma_start(cind_sb[:], cind_ap)

            tT = tpool.tile([P, ER // P, S], BF16, name="tT")
            pr_t = prp.tile([E, 512], F32, name="pr_t")
            ones8 = rpool.tile([E, 1], F32, name="ones8")
            nc.any.memset(ones8[:], 1.0)
            ones_row = rpool.tile([1, P], F32, name="ones_row")
            nc.any.memset(ones_row[:], 1.0)
            coeff_cols = rpool.tile([P, ER // P], F32, name="coeff_cols")

            SC = S // 512  # 4 s-chunks

            for p in range(NP):
                i0 = p * (IN // NP)
                # --- phase-T weights first: the very first matmul needs ldc0 ---
                def load_ldc(j):
                    ldcj = spool.tile([P, 8, ER], F32R, tag="stream", name=f"ldc_{p}_{j}")
                    nc.sync.dma_start(
                        ldcj[:],
                        ldn_ap[i0 + j * 1024 : i0 + (j + 1) * 1024, :].rearrange(
                            "(io pp) e -> pp io e", pp=P
                        ),
                    )
                    return ldcj

                ldcs = [load_ldc(0)]

                # --- load x in eighths (each [256 i, S]) for DMA-queue
                # parallelism and early phase-T start ---
                xq = []
                for k in range(8):
                    xqk = xpool.tile([P, 2, S], F32R, tag=f"xq{k}", name=f"xq{k}_{p}")
                    # alternate SWDGE/HWDGE so x streams over both DMA paths;
                    # xq0 goes on gpsimd so it loads in parallel with ldc0 (sync)
                    eng = nc.gpsimd if k % 2 == 0 else nc.sync
                    eng.dma_start(
                        xqk[:],
                        xT_ap[i0 + k * 256 : i0 + (k + 1) * 256, :].rearrange(
                            "(io pp) s -> pp io s", pp=P
                        ),
                    )
                    xq.append(xqk)

                def xs(io, fslice):
                    return xq[io // 2][:, io % 2, fslice]

                ldcs.append(load_ldc(1))  # needed only at phase-T midpoint

                # --- phase T: tT += (ldn_pass.T @ x_pass), chunked by 8 io ---
                for j in range(2):
                    ldc = ldcs[j]
                    for u in range(ER // P):
                        for c in range(SC):
                            ps = ptp.tile([P, 512], F32, tag="pt", name=f"pt_{p}_{j}_{u}_{c}")
                            for jo in range(8):
                                io = j * 8 + jo
                                nc.tensor.matmul(
                                    ps[:],
                                    ldc[:, jo, u * P : (u + 1) * P],
                                    xs(io, slice(c * 512, (c + 1) * 512)),
                                    start=(jo == 0),
                                    stop=(jo == 7),
                                )
                            dst = tT[:, u, c * 512 : (c + 1) * 512]
                            if p == 0 and j == 0:
                                nc.vector.tensor_copy(dst, ps[:])
                            else:
                                nc.vector.tensor_tensor(
                                    dst, dst, ps[:], mybir.AluOpType.add
                                )

                # --- router projection: pr_t[e, j] += sum_i x[s,i] mk[i,e] ---
                for c in range(SC):
                    for io in range(IOP):
                        nc.tensor.matmul(
                            pr_t[:],
                            mk_sb[:, p * IOP + io, :],
                            xs(io, slice(c * 512, (c + 1) * 512)),
                            start=(p == 0 and c == 0 and io == 0),
                            stop=(p == NP - 1 and c == SC - 1 and io == IOP - 1),
                        )

                if p == NP - 1:
                    # --- router finalize (on-device softmax) ---
                    scores = rpool.tile([E, 1], F32, name="scores")
                    nc.vector.reduce_sum(scores[:], pr_t[:], axis=mybir.AxisListType.X)
                    exps = rpool.tile([E, 1], F32, name="exps")
                    nc.scalar.activation(
                        exps[:], scores[:], mybir.ActivationFunctionType.Exp
                    )
                    # sum(exp) via PE partition reduction -> [1, 1]
                    psum_s = pccp.tile([1, 1], F32, tag="pcc", name="psum_s")
                    nc.tensor.matmul(psum_s[:], exps[:], ones8[:], start=True, stop=True)
                    rinv = rpool.tile([1, 1], F32, name="rinv")
                    nc.vector.reciprocal(rinv[:], psum_s[:])
                    # broadcast 1/sum to all 128 partitions via rank-1 outer product
                    rb_p = pccp.tile([P, 1], F32, tag="pcc", name="rb_p")
                    nc.tensor.matmul(rb_p[:], ones_row[:], rinv[:], start=True, stop=True)
                    rb = rpool.tile([P, 1], F32, name="rb")
                    nc.vector.tensor_copy(rb[:], rb_p[:])
                    # partition placement: cc_un[pp, u] = exp(score[(u*128+pp)//64])
                    cc_un = rpool.tile([P, ER // P], F32, name="cc_un")
                    for u in range(ER // P):
                        pcc = pccp.tile([P, 1], F32, tag="pcc", name=f"pcc_{u}")
                        nc.tensor.matmul(
                            pcc[:],
                            cind_sb[:, u * P : (u + 1) * P],
                            exps[:],
                            start=True,
                            stop=True,
                        )
                        nc.vector.tensor_copy(cc_un[:, u : u + 1], pcc[:])
                    # coeff_cols = cc_un / sum(exp) * SCALE  (SCALE == 1.0)
                    nc.vector.tensor_scalar_mul(coeff_cols[:], cc_un[:], rb[:])

                lsc_tiles = [None] * (OH // 512)

                def load_lsc(nn_):
                    lraw = spool.tile(
                        [P, ER // P, 512], F32, tag="stream", name=f"lraw_{nn_}"
                    )
                    nc.gpsimd.dma_start(
                        lraw[:],
                        lup_ap[:, nn_ * 512 : (nn_ + 1) * 512].rearrange(
                            "(u pp) o -> pp u o", pp=P
                        ),
                    )
                    t = lpool.tile([P, ER // P, 512], BF16, tag="lsc", name=f"lsc_{nn_}")
                    nc.vector.tensor_tensor(
                        t[:],
                        lraw[:],
                        coeff_cols[:, :, None].to_broadcast((P, ER // P, 512)),
                        mybir.AluOpType.mult,
                    )
                    lsc_tiles[nn_] = t

                if p == NP - 1:
                    load_lsc(0)

                # --- main output loop ---
                for n in range(OH // 512):
                    wc = []
                    for j in range(2):
                        wcj = spool.tile(
                            [P, 8, 512], F32R, tag="stream", name=f"wc_{p}_{n}_{j}"
                        )
                        (nc.sync if j == 0 else nc.gpsimd).dma_start(
                            wcj[:],
                            wT_ap[
                                i0 + j * 1024 : i0 + (j + 1) * 1024,
                                n * 512 : (n + 1) * 512,
                            ].rearrange("(io pp) o -> pp io o", pp=P),
                        )
                        wc.append(wcj)
                    if p == NP - 1:
                        lsc = lsc_tiles[n]
                        if n + 1 < OH // 512:
                            # prefetch next n's scaled lup during this m-loop
                            load_lsc(n + 1)

                    for m in range(S // P):
                        sl = (
                            slice(m * P, (m + 1) * P),
                            slice(n * 512, (n + 1) * 512),
                        )
                        if p == NP - 1:
                            oprev = opool.tile(
                                [P, 512], F32, tag="oprev", name=f"opr_{n}_{m}", bufs=1
                            )
                            nc.sync.dma_start(oprev[:], out_p1_ap[sl])
                        po_t = pop.tile([P, 512], F32, tag="po", name=f"po_{p}_{n}_{m}")
                        for io in range(IOP):
                            nc.tensor.matmul(
                                po_t[:],
                                xs(io, slice(m * P, (m + 1) * P)),
                                wc[io // 8][:, io % 8, :],
                                start=(io == 0),
                                stop=(io == IOP - 1 and p < NP - 1),
                            )
                        if p == NP - 1:
                            for u in range(ER // P):
                                nc.tensor.matmul(
                                    po_t[:],
                                    tT[:, u, m * P : (m + 1) * P],
                                    lsc[:, u, :],
                                    start=False,
                                    stop=(u == ER // P - 1),
                                )
                        ost = opool.tile(
                            [P, 512], F32, tag="ost", name=f"ost_{p}_{n}_{m}", bufs=1
                        )
                        if p == 0:
                            nc.vector.tensor_copy(ost[:], po_t[:])
                            (nc.sync if m % 2 == 0 else nc.gpsimd).dma_start(
                                out_p1_ap[sl], ost[:]
                            )
                        else:
                            nc.vector.tensor_tensor(
                                ost[:], oprev[:], po_t[:], mybir.AluOpType.add
                            )
                            (nc.sync if m % 2 == 0 else nc.gpsimd).dma_start(
                                out_ap[sl], ost[:]
                            )

    nc.compile()
    return nc


def kernel(x, W_org, lora_down, lora_up, lora_keys, Wq):
    x = np.ascontiguousarray(np.asarray(x, dtype=np.float32))
    W_org = np.asarray(W_org, dtype=np.float32)
    lora_down = np.asarray(lora_down, dtype=np.float32)
    lora_up = np.asarray(lora_up, dtype=np.float32)
    lora_keys = np.asarray(lora_keys, dtype=np.float32)
    Wq = np.asarray(Wq, dtype=np.float32)

    # Host-side constant folding / layout prep (transposes to K-major)
    wT_full = np.ascontiguousarray(W_org.T)                          # [IN, OUT]
    ldn = np.ascontiguousarray(lora_down.transpose(1, 0, 2).reshape(IN, ER))
    lup_full = np.ascontiguousarray(lora_up.reshape(ER, OUT))
    mk = np.ascontiguousarray(
        (Wq.T @ lora_keys.T) / (S * math.sqrt(K))
    ).astype(np.float32)                                             # [IN, E]
    cind = np.repeat(np.eye(E, dtype=np.float32), R, axis=1)         # [E, ER]
    xT = [np.ascontiguousarray(x[b].T) for b in range(B)]            # [IN, S]
    wT_half = [np.ascontiguousarray(wT_full[:, h * OH : (h + 1) * OH]) for h in range(2)]
    lup_half = [np.ascontiguousarray(lup_full[:, h * OH : (h + 1) * OH]) for h in range(2)]

    if "nc" not in _NC_CACHE:
        _NC_CACHE["nc"] = _build_nc()
    nc = _NC_CACHE["nc"]

    in_maps = []
    for c in range(8):
        b, h = c // 2, c % 2
        in_maps.append(
            {
                "xT": xT[b],
                "wT": wT_half[h],
                "ldn": ldn,
                "lup": lup_half[h],
                "mk": mk,
                "cind": cind,
            }
        )

    res = run_bass_kernel_spmd(nc, in_maps, core_ids=list(range(8)), trace=False)
    _NC_CACHE["last_result"] = res
    _NC_CACHE["last_in_maps"] = in_maps

    outp = np.empty((B, S, OUT), dtype=np.float32)
    for c in range(8):
        b, h = c // 2, c % 2
        outp[b, :, h * OH : (h + 1) * OH] = res.results[c]["out"]
    return outp


def _build_baseline_nc():
    """Same I/O signature as the real kernel, near-zero device work.

    Used to measure the fixed dispatch/relay overhead of one execution so the
    real kernel's device time can be estimated as (full - baseline)."""
    nc = bacc.Bacc("TRN2", target_bir_lowering=False, debug=False)
    xT = nc.dram_tensor("xT", [IN, S], F32R, kind="ExternalInput")
    wT = nc.dram_tensor("wT", [IN, OH], F32R, kind="ExternalInput")
    ldn = nc.dram_tensor("ldn", [IN, ER], F32R, kind="ExternalInput")
    lup = nc.dram_tensor("lup", [ER, OH], F32, kind="ExternalInput")
    mk = nc.dram_tensor("mk", [IN, E], F32R, kind="ExternalInput")
    cind = nc.dram_tensor("cind", [E, ER], F32, kind="ExternalInput")
    out = nc.dram_tensor("out", [S, OH], F32, kind="ExternalOutput")
    with tile.TileContext(nc) as tc:
        with tc.tile_pool(name="bp", bufs=1) as bp:
            t = bp.tile([P, 512], F32, name="t")
            nc.sync.dma_start(t[:], lup.ap()[:P, :512])
            nc.sync.dma_start(out.ap()[:P, :512], t[:])
            _ = (xT, wT, ldn, mk, cind)
    nc.compile()
    return nc


def benchmark_baseline(iters: int = 8):
    if "bnc" not in _NC_CACHE:
        _NC_CACHE["bnc"] = _build_baseline_nc()
    return benchmark(iters, nc=_NC_CACHE["bnc"])


def benchmark(iters: int = 8, nc=None):
    """Time device execution with inputs pre-placed on the 8 cores.

    Mirrors bass2jax.run_bass_via_pjrt's multi-core shard_map path but keeps
    the non-donated inputs resident on device so the timed region is
    dispatch + NEFF execution only.  Returns per-iteration seconds.
    """
    import time

    import jax
    from jax.experimental.shard_map import shard_map
    from jax.sharding import Mesh, NamedSharding, PartitionSpec

    from concourse import bass2jax, mybir as _mybir

    if nc is None:
        nc = _NC_CACHE["nc"]
    in_maps = _NC_CACHE["last_in_maps"]
    n_cores = len(in_maps)

    bass2jax.install_neuronx_cc_hook()

    partition_name = nc.partition_id_tensor.name if nc.partition_id_tensor else None
    in_names, out_names, out_avals, zero_outs = [], [], [], []
    for alloc in nc.m.functions[0].allocations:
        if not isinstance(alloc, _mybir.MemoryLocationSet):
            continue
        name = alloc.memorylocations[0].name
        if alloc.kind == "ExternalInput":
            if name != partition_name:
                in_names.append(name)
        elif alloc.kind == "ExternalOutput":
            aval = jax.core.ShapedArray(
                tuple(alloc.tensor_shape), _mybir.dt.np(alloc.dtype)
            )
            out_avals.append(aval)
            out_names.append(name)
            zero_outs.append(np.zeros(aval.shape, aval.dtype))
    n_params = len(in_names)
    n_outs = len(out_avals)
    all_in_names = in_names + out_names
    if partition_name is not None:
        all_in_names = all_in_names + [partition_name]

    def _body(*args):
        operands = list(args)
        if partition_name is not None:
            operands.append(bass2jax.partition_id_tensor())
        outs = bass2jax._bass_exec_p.bind(
            *operands,
            out_avals=tuple(out_avals),
            in_names=tuple(all_in_names),
            out_names=tuple(out_names),
            lowering_input_output_aliases=(),
            sim_require_finite=True,
            sim_require_nnan=True,
            nc=nc,
        )
        return tuple(outs)

    _body.__name__ = "_body"

    devices = jax.devices()[:n_cores]
    mesh = Mesh(np.asarray(devices), ("core",))
    spec = PartitionSpec("core")
    sharding = NamedSharding(mesh, spec)
    donate = tuple(range(n_params, n_params + n_outs))
    fn = jax.jit(
        shard_map(
            _body,
            mesh=mesh,
            in_specs=(spec,) * (n_params + n_outs),
            out_specs=(spec,) * n_outs,
            check_rep=False,
        ),
        donate_argnums=donate,
        keep_unused=True,
    )

    concat_in = [
        np.concatenate([np.asarray(in_maps[c][nm]) for c in range(n_cores)], axis=0)
        for nm in in_names
    ]
    concat_zero = [
        np.zeros((n_cores * z.shape[0], *z.shape[1:]), z.dtype) for z in zero_outs
    ]
    dev_in = [jax.device_put(a, sharding) for a in concat_in]
    for a in dev_in:
        a.block_until_ready()

    times = []
    for _ in range(iters + 1):
        dev_zero = [jax.device_put(z, sharding) for z in concat_zero]
        for z in dev_zero:
            z.block_until_ready()
        t0 = time.perf_counter()
        outs = fn(*dev_in, *dev_zero)
        for o in outs:
            o.block_until_ready()
        times.append(time.perf_counter() - t0)
    return times[1:]  # drop warmup/compile call



# revision 5
# speedup vs baseline: 39.7146x; 39.7146x over previous
"""AttLoRA MoE-routing kernel for 8 Trainium2 NeuronCores.

Reference computation (per problem nn_AttLoRAModule_85839216378078):
    base  = x @ W_org.T                                    [B,S,OUT]
    q     = x.mean(axis=1) @ Wq.T                          [B,K]
    coef  = softmax(q @ lora_keys.T / sqrt(K))             [B,E]
    h     = x @ lora_down[e]                               [B,S,E,R]
    delta = sum_e coef[b,e] * (h[...,e,:] @ lora_up[e])    [B,S,OUT]
    out   = base + delta * SCALE
(SCALE = 1.0 for this problem instance.)

Sharding: 8 cores = 4 batches x 2 sequence-halves.  Core c handles batch
c//2, tokens [(c%2)*1024, (c%2+1)*1024), all OUT columns.  Sequence split
(instead of OUT split) halves the per-core x@lora_down work: each core
computes tT only for its own 1024 tokens.

Device strategy (per core), all heavy matmuls in bf16 (1 PE cycle/row,
same as f32r, but half the SBUF/DMA and single-pass-capable):
  - Single contraction pass: x own-half [4096, 1024] bf16 stays fully
    SBUF-resident (64 KiB/partition); base + delta accumulate in one PSUM
    group per (n, m) output tile; one eviction, no DRAM round-trip.
  - T-phase: tT[er, s] = (lora_down.T @ x) for all 8 experts over own
    tokens; 8 PSUM groups of 32 matmuls.
  - Router: needs sum over the FULL sequence of x.  qsum[i] = sum_s x[i,s]
    is computed on the Vector engine (resident own half + streamed other
    half), then scores = qsum @ mk via 32 rank-1 f32 matmuls; softmax on
    one partition; coefficients broadcast to partitions via indicator
    matmuls (cind) and folded into bf16-scaled lora_up tiles (lsc).
  - PE work/core ~= 1.31M moving-rows ~= 546 us at warm 2.4 GHz clock.
"""

import math
import os

import numpy as np
import ml_dtypes

import concourse.bacc as bacc
import concourse.mybir as mybir
import concourse.tile as tile
from concourse.bass_utils import run_bass_kernel_spmd

# Problem shapes (hardcoded per contest contract)
B, S, IN, OUT = 4, 2048, 4096, 4096
E, R, K = 8, 64, 128
ER = E * R            # 512
SH = S // 2           # 1024 tokens per core
P = 128
NIO = IN // P         # 32 contraction subtiles
NN = OUT // 512       # 8 output column blocks
NM = SH // P          # 8 token row blocks
SCALE = 1.0           # (alpha/lora_dim) * multiplier

F32 = mybir.dt.float32
BF16 = mybir.dt.bfloat16
BF16_NP = ml_dtypes.bfloat16

_NC_CACHE = {}


def _build_nc():
    nc = bacc.Bacc("TRN2", target_bir_lowering=False, debug=False)

    xh = nc.dram_tensor("xh", [IN, SH], BF16, kind="ExternalInput")
    xo = nc.dram_tensor("xo", [IN, SH], BF16, kind="ExternalInput")
    wT = nc.dram_tensor("wT", [IN, OUT], BF16, kind="ExternalInput")
    ldn = nc.dram_tensor("ldn", [IN, ER], BF16, kind="ExternalInput")
    lup = nc.dram_tensor("lup", [ER, OUT], BF16, kind="ExternalInput")
    mk = nc.dram_tensor("mk", [IN, E], F32, kind="ExternalInput")
    cind = nc.dram_tensor("cind", [E, ER], F32, kind="ExternalInput")
    out = nc.dram_tensor("out", [SH, OUT], F32, kind="ExternalOutput")

    xh_ap, xo_ap, wT_ap, ldn_ap, lup_ap, mk_ap, cind_ap, out_ap = (
        t.ap() for t in (xh, xo, wT, ldn, lup, mk, cind, out)
    )

    trace_sim = os.environ.get("KERNEL_SIM_TRACE", "0") == "1"
    with tile.TileContext(nc, trace_sim=trace_sim) as tc:
        with (
            tc.tile_pool(name="xpool", bufs=1) as xpool,
            tc.tile_pool(name="xopool", bufs=2) as xopool,
            tc.tile_pool(name="spool", bufs=5) as spool,
            tc.tile_pool(name="tpool", bufs=1) as tpool,
            tc.tile_pool(name="lrpool", bufs=2) as lrpool,
            tc.tile_pool(name="lpool", bufs=2) as lpool,
            tc.tile_pool(name="opool", bufs=2) as opool,
            tc.tile_pool(name="rpool", bufs=1) as rpool,
            tc.tile_pool(name="ptp", bufs=2, space="PSUM") as ptp,
            tc.tile_pool(name="pop", bufs=4, space="PSUM") as pop,
            tc.tile_pool(name="prp", bufs=1, space="PSUM") as prp,
            tc.tile_pool(name="pccp", bufs=1, space="PSUM") as pccp,
        ):
            # --- persistent small tiles ---
            mk_sb = rpool.tile([P, NIO, E], F32, name="mk_sb")
            nc.sync.dma_start(mk_sb[:], mk_ap.rearrange("(io pp) e -> pp io e", pp=P))
            cind_sb = rpool.tile([E, ER], F32, name="cind_sb")
            nc.sync.dma_start(cind_sb[:], cind_ap)
            ones8 = rpool.tile([E, 1], F32, name="ones8")
            nc.any.memset(ones8[:], 1.0)
            ones_row = rpool.tile([1, P], F32, name="ones_row")
            nc.any.memset(ones_row[:], 1.0)

            tT = tpool.tile([P, 4, SH], BF16, name="tT")
            qsh = rpool.tile([P, NIO], F32, name="qsh")
            qso = rpool.tile([P, NIO], F32, name="qso")
            qsum = rpool.tile([P, NIO], F32, name="qsum")

            # --- input streams ---
            # x own half: resident, alternating DMA queues for bandwidth
            xq = []
            for k in range(8):
                t = xpool.tile([P, 4, SH], BF16, name=f"xh{k}")
                eng = nc.sync if k % 2 == 0 else nc.gpsimd
                eng.dma_start(
                    t[:],
                    xh_ap[k * 512 : (k + 1) * 512, :].rearrange(
                        "(io pp) s -> pp io s", pp=P
                    ),
                )
                xq.append(t)

            def xs(io, fslice):
                return xq[io // 4][:, io % 4, fslice]

            # lora_down: 2 chunks through the shared stream pool
            ldcs = []
            for j in range(2):
                ldc = spool.tile([P, 16, ER], BF16, tag="stream", name=f"ldc{j}")
                (nc.sync if j == 0 else nc.gpsimd).dma_start(
                    ldc[:],
                    ldn_ap[j * 2048 : (j + 1) * 2048, :].rearrange(
                        "(io pp) e -> pp io e", pp=P
                    ),
                )
                ldcs.append(ldc)

            # W n=0 block prefetch (2 chunks); later blocks stream in-loop
            def load_wc(n):
                wcs = []
                for j in range(2):
                    wcj = spool.tile(
                        [P, 16, 512], BF16, tag="stream", name=f"wc_{n}_{j}"
                    )
                    (nc.sync if j == 0 else nc.gpsimd).dma_start(
                        wcj[:],
                        wT_ap[
                            j * 2048 : (j + 1) * 2048, n * 512 : (n + 1) * 512
                        ].rearrange("(io pp) o -> pp io o", pp=P),
                    )
                    wcs.append(wcj)
                return wcs

            wc_tiles = {0: load_wc(0)}

            # x other half: streamed for the router qsum only
            xoq = []
            for k in range(8):
                t = xopool.tile([P, 4, SH], BF16, tag="xo", name=f"xo{k}")
                eng = nc.gpsimd if k % 2 == 0 else nc.sync
                eng.dma_start(
                    t[:],
                    xo_ap[k * 512 : (k + 1) * 512, :].rearrange(
                        "(io pp) s -> pp io s", pp=P
                    ),
                )
                xoq.append(t)

            # --- router qsum on DVE (overlaps the T-phase on PE) ---
            # own half first (arrives first; DVE order matters: tT
            # evictions are queued behind these)
            for k in range(8):
                nc.vector.reduce_sum(
                    qsh[:, 4 * k : 4 * k + 4], xq[k][:], axis=mybir.AxisListType.X
                )

            # --- T phase: tT[er, s] = ldn.T @ x (own half) ---
            for g, (u, cc) in enumerate(
                (u, cc) for u in range(4) for cc in range(2)
            ):
                ps = ptp.tile([P, 512], F32, tag="pt", name=f"pt_{u}_{cc}")
                for io in range(NIO):
                    nc.tensor.matmul(
                        ps[:],
                        ldcs[io // 16][:, io % 16, u * P : (u + 1) * P],
                        xs(io, slice(cc * 512, (cc + 1) * 512)),
                        start=(io == 0),
                        stop=(io == NIO - 1),
                    )
                nc.vector.tensor_copy(tT[:, u, cc * 512 : (cc + 1) * 512], ps[:])
                # interleave other-half qsum reduces into the DVE queue
                if g < 4:
                    nc.vector.reduce_sum(
                        qso[:, 8 * g : 8 * g + 4],
                        xoq[2 * g][:],
                        axis=mybir.AxisListType.X,
                    )
                    nc.vector.reduce_sum(
                        qso[:, 8 * g + 4 : 8 * g + 8],
                        xoq[2 * g + 1][:],
                        axis=mybir.AxisListType.X,
                    )
                if g == 4:
                    nc.vector.tensor_tensor(
                        qsum[:], qsh[:], qso[:], mybir.AluOpType.add
                    )

            # --- router scores on PE (tiny): pr[e] = sum_io mk.T @ qsum ---
            pr = prp.tile([E, 1], F32, tag="pr", name="pr")
            for io in range(NIO):
                nc.tensor.matmul(
                    pr[:],
                    mk_sb[:, io, :],
                    qsum[:, io : io + 1],
                    start=(io == 0),
                    stop=(io == NIO - 1),
                )

            # --- softmax + coefficient placement ---
            scores = rpool.tile([E, 1], F32, name="scores")
            nc.vector.tensor_copy(scores[:], pr[:])
            exps = rpool.tile([E, 1], F32, name="exps")
            nc.scalar.activation(exps[:], scores[:], mybir.ActivationFunctionType.Exp)
            psum_s = pccp.tile([1, 1], F32, tag="pcc", name="psum_s")
            nc.tensor.matmul(psum_s[:], exps[:], ones8[:], start=True, stop=True)
            rinv = rpool.tile([1, 1], F32, name="rinv")
            nc.vector.reciprocal(rinv[:], psum_s[:])
            rb_p = pccp.tile([P, 1], F32, tag="pcc", name="rb_p")
            nc.tensor.matmul(rb_p[:], ones_row[:], rinv[:], start=True, stop=True)
            rb = rpool.tile([P, 1], F32, name="rb")
            nc.vector.tensor_copy(rb[:], rb_p[:])
            # cc_un[pp, u] = exp(score[(u*128+pp)//64])
            cc_un = rpool.tile([P, 4], F32, name="cc_un")
            for u in range(4):
                pcc = pccp.tile([P, 1], F32, tag="pcc", name=f"pcc_{u}")
                nc.tensor.matmul(
                    pcc[:],
                    cind_sb[:, u * P : (u + 1) * P],
                    exps[:],
                    start=True,
                    stop=True,
                )
                nc.vector.tensor_copy(cc_un[:, u : u + 1], pcc[:])
            coeff_f = rpool.tile([P, 4], F32, name="coeff_f")
            nc.vector.tensor_scalar_mul(coeff_f[:], cc_un[:], rb[:])
            coeff = rpool.tile([P, 4], BF16, name="coeff")
            nc.vector.tensor_copy(coeff[:], coeff_f[:])

            # --- coefficient-scaled lora_up tiles ---
            lsc_tiles = [None] * NN

            def load_lsc(n):
                lraw = lrpool.tile([P, 4, 512], BF16, tag="lraw", name=f"lraw_{n}")
                nc.gpsimd.dma_start(
                    lraw[:],
                    lup_ap[:, n * 512 : (n + 1) * 512].rearrange(
                        "(u pp) o -> pp u o", pp=P
                    ),
                )
                t = lpool.tile([P, 4, 512], BF16, tag="lsc", name=f"lsc_{n}")
                nc.vector.tensor_tensor(
                    t[:],
                    lraw[:],
                    coeff[:, :, None].to_broadcast((P, 4, 512)),
                    mybir.AluOpType.mult,
                )
                lsc_tiles[n] = t

            load_lsc(0)

            # --- main loop: out = x @ W + tT.T @ lsc, fused in PSUM ---
            for n in range(NN):
                if n + 1 < NN:
                    wc_tiles[n + 1] = load_wc(n + 1)
                wc = wc_tiles.pop(n)
                lsc = lsc_tiles[n]
                for m in range(NM):
                    sl = (
                        slice(m * P, (m + 1) * P),
                        slice(n * 512, (n + 1) * 512),
                    )
                    po = pop.tile([P, 512], F32, tag="po", name=f"po_{n}_{m}")
                    for io in range(NIO):
                        nc.tensor.matmul(
                            po[:],
                            xs(io, slice(m * P, (m + 1) * P)),
                            wc[io // 16][:, io % 16, :],
                            start=(io == 0),
                            stop=False,
                        )
                    for u in range(4):
                        nc.tensor.matmul(
                            po[:],
                            tT[:, u, m * P : (m + 1) * P],
                            lsc[:, u, :],
                            start=False,
                            stop=(u == 3),
                        )
                    ost = opool.tile([P, 512], F32, tag="ost", name=f"ost_{n}_{m}")
                    nc.vector.tensor_copy(ost[:], po[:])
                    (nc.sync if m % 2 == 0 else nc.gpsimd).dma_start(
                        out_ap[sl], ost[:]
                    )
                    if m == 0 and n + 1 < NN:
                        # prefetch next block's scaled lora_up after the
                        # first eviction so a late lraw DMA cannot stall
                        # the DVE queue ahead of this block's evictions
                        load_lsc(n + 1)

    nc.compile()
    return nc


def kernel(x, W_org, lora_down, lora_up, lora_keys, Wq):
    x = np.asarray(x, dtype=np.float32)
    W_org = np.asarray(W_org, dtype=np.float32)
    lora_down = np.asarray(lora_down, dtype=np.float32)
    lora_up = np.asarray(lora_up, dtype=np.float32)
    lora_keys = np.asarray(lora_keys, dtype=np.float32)
    Wq = np.asarray(Wq, dtype=np.float32)

    # Host-side constant folding / layout prep (transposes to K-major)
    wT = np.ascontiguousarray(W_org.T).astype(BF16_NP)               # [IN, OUT]
    ldn = np.ascontiguousarray(
        lora_down.transpose(1, 0, 2).reshape(IN, ER)
    ).astype(BF16_NP)                                                # [IN, ER]
    lup = np.ascontiguousarray(lora_up.reshape(ER, OUT)).astype(BF16_NP)
    mk = np.ascontiguousarray(
        (Wq.T @ lora_keys.T) / (S * math.sqrt(K))
    ).astype(np.float32)                                             # [IN, E]
    cind = np.repeat(np.eye(E, dtype=np.float32), R, axis=1)         # [E, ER]
    xT = [np.ascontiguousarray(x[b].T).astype(BF16_NP) for b in range(B)]

    if "nc" not in _NC_CACHE:
        _NC_CACHE["nc"] = _build_nc()
    nc = _NC_CACHE["nc"]

    in_maps = []
    for c in range(8):
        b, h = c // 2, c % 2
        in_maps.append(
            {
                "xh": np.ascontiguousarray(xT[b][:, h * SH : (h + 1) * SH]),
                "xo": np.ascontiguousarray(xT[b][:, (1 - h) * SH : (2 - h) * SH]),
                "wT": wT,
                "ldn": ldn,
                "lup": lup,
                "mk": mk,
                "cind": cind,
            }
        )

    res = run_bass_kernel_spmd(nc, in_maps, core_ids=list(range(8)), trace=False)
    _NC_CACHE["last_result"] = res
    _NC_CACHE["last_in_maps"] = in_maps

    outp = np.empty((B, S, OUT), dtype=np.float32)
    for c in range(8):
        b, h = c // 2, c % 2
        outp[b, h * SH : (h + 1) * SH, :] = res.results[c]["out"]
    return outp


def _benchmark_fn(nc, in_maps, chain):
    """Build a jitted dispatcher that executes the NEFF `chain` times
    back-to-back on-device (output buffer threaded through as the donated
    out operand of the next iteration, forcing serialization)."""
    import jax
    from jax.experimental.shard_map import shard_map
    from jax.sharding import Mesh, NamedSharding, PartitionSpec

    from concourse import bass2jax, mybir as _mybir

    bass2jax.install_neuronx_cc_hook()

    n_cores = len(in_maps)
    partition_name = nc.partition_id_tensor.name if nc.partition_id_tensor else None
    in_names, out_names, out_avals, zero_outs = [], [], [], []
    for alloc in nc.m.functions[0].allocations:
        if not isinstance(alloc, _mybir.MemoryLocationSet):
            continue
        name = alloc.memorylocations[0].name
        if alloc.kind == "ExternalInput":
            if name != partition_name:
                in_names.append(name)
        elif alloc.kind == "ExternalOutput":
            aval = jax.core.ShapedArray(
                tuple(alloc.tensor_shape), _mybir.dt.np(alloc.dtype)
            )
            out_avals.append(aval)
            out_names.append(name)
            zero_outs.append(np.zeros(aval.shape, aval.dtype))
    n_params = len(in_names)
    n_outs = len(out_avals)
    all_in_names = in_names + out_names
    if partition_name is not None:
        all_in_names = all_in_names + [partition_name]

    def _exec_once(ins, outs):
        operands = list(ins) + list(outs)
        if partition_name is not None:
            operands.append(bass2jax.partition_id_tensor())
        return bass2jax._bass_exec_p.bind(
            *operands,
            out_avals=tuple(out_avals),
            in_names=tuple(all_in_names),
            out_names=tuple(out_names),
            lowering_input_output_aliases=(),
            sim_require_finite=True,
            sim_require_nnan=True,
            nc=nc,
        )

    def _body(*args):
        ins = args[:n_params]
        outs = list(args[n_params:])
        for _ in range(chain):
            outs = list(_exec_once(ins, outs))
        return tuple(outs)

    _body.__name__ = "_body"

    devices = jax.devices()[:n_cores]
    mesh = Mesh(np.asarray(devices), ("core",))
    spec = PartitionSpec("core")
    sharding = NamedSharding(mesh, spec)
    donate = tuple(range(n_params, n_params + n_outs))
    fn = jax.jit(
        shard_map(
            _body,
            mesh=mesh,
            in_specs=(spec,) * (n_params + n_outs),
            out_specs=(spec,) * n_outs,
            check_rep=False,
        ),
        donate_argnums=donate,
        keep_unused=True,
    )

    concat_in = [
        np.concatenate([np.asarray(in_maps[c][nm]) for c in range(n_cores)], axis=0)
        for nm in in_names
    ]
    concat_zero = [
        np.zeros((n_cores * z.shape[0], *z.shape[1:]), z.dtype) for z in zero_outs
    ]
    dev_in = [jax.device_put(a, sharding) for a in concat_in]
    for a in dev_in:
        a.block_until_ready()
    return fn, dev_in, concat_zero, sharding


def benchmark_chained(iters=12, chain_hi=17, chain_lo=1, nc=None):
    """Estimate per-execution device time as
    (min t(chain_hi) - min t(chain_lo)) / (chain_hi - chain_lo).
    Chaining many executions per dispatch amortizes the (noisy, tens-of-ms)
    axon relay overhead that a single dispatch cannot distinguish from
    device time."""
    import time
    import jax

    if nc is None:
        nc = _NC_CACHE["nc"]
    in_maps = _NC_CACHE["last_in_maps"]

    results = {}
    for chain in (chain_lo, chain_hi):
        fn, dev_in, concat_zero, sharding = _benchmark_fn(nc, in_maps, chain)
        times = []
        for it in range(iters + 1):
            import jax as _jax

            dev_zero = [_jax.device_put(z, sharding) for z in concat_zero]
            for z in dev_zero:
                z.block_until_ready()
            t0 = time.perf_counter()
            outs = fn(*dev_in, *dev_zero)
            for o in outs:
                o.block_until_ready()
            times.append(time.perf_counter() - t0)
        results[chain] = times[1:]
    per_exec = (min(results[chain_hi]) - min(results[chain_lo])) / (
        chain_hi - chain_lo
    )
    return per_exec, results


def benchmark(iters: int = 8, nc=None):
    """Single-dispatch wall times (relay noise included); kept for
    comparison with earlier measurements."""
    import time
    import jax

    if nc is None:
        nc = _NC_CACHE["nc"]
    in_maps = _NC_CACHE["last_in_maps"]
    fn, dev_in, concat_zero, sharding = _benchmark_fn(nc, in_maps, 1)
    times = []
    for _ in range(iters + 1):
        dev_zero = [jax.device_put(z, sharding) for z in concat_zero]
        for z in dev_zero:
            z.block_until_ready()
        t0 = time.perf_counter()
        outs = fn(*dev_in, *dev_zero)
        for o in outs:
            o.block_until_ready()
        times.append(time.perf_counter() - t0)
    return times[1:]


# revision 8
# speedup vs baseline: 43.2385x; 1.0887x over previous
"""AttLoRA MoE-routing kernel for 8 Trainium2 NeuronCores.

Reference computation (per problem nn_AttLoRAModule_85839216378078):
    base  = x @ W_org.T                                    [B,S,OUT]
    q     = x.mean(axis=1) @ Wq.T                          [B,K]
    coef  = softmax(q @ lora_keys.T / sqrt(K))             [B,E]
    h     = x @ lora_down[e]                               [B,S,E,R]
    delta = sum_e coef[b,e] * (h[...,e,:] @ lora_up[e])    [B,S,OUT]
    out   = base + delta * SCALE        (SCALE = 1.0 here)

Sharding: 8 cores = 4 batches x 2 sequence-halves.  Core c handles batch
c//2, tokens [(c%2)*1024, (c%2+1)*1024), all OUT columns.  The sequence
split (instead of an OUT split) halves the per-core x@lora_down work.

Device strategy (per core):
  - base GEMM in bf16 (1 PE cycle/row, = f32r rate, half the SBUF/DMA):
    x own-half [4096,1024] bf16 SBUF-resident; 64 PSUM groups of
    32 base matmuls + 2 fp8-DoubleRow delta matmuls; single eviction.
  - LoRA T-phase (x @ lora_down) and delta (tT.T @ lsc) in fp8e4m3 with
    perf_mode=DoubleRow (2 contraction rows/cycle): 131k -> 33k PE
    cycles each.  Host prescales lora_down/lora_up by 32 to dodge fp8
    subnormals; the tT eviction rescales by 2^-10.
  - Router entirely on PE in fp8 DoubleRow: pr[e,j] accumulates
    x[:,s] @ (Wq.T@keys.T) over BOTH sequence halves (other half
    streamed as fp8), then reduce + softmax on one partition and
    coefficients broadcast via indicator (cind) matmuls.  Coefficients
    are folded into bf16 lora_up tiles -> fp8 lsc.
  - PE work/core ~= 1.12M cycles ~= 468 us at warm 2.4 GHz.

The `reps` build parameter emits the whole computation N times in one
NEFF (tile tags shared across reps so SBUF slots rotate) — used only for
benchmarking: per-exec time = (t(reps_hi) - t(1)) / (reps_hi - 1), which
cancels the tens-of-ms axon relay dispatch noise.
"""

import math
import os

import numpy as np
import ml_dtypes

import concourse.bacc as bacc
import concourse.mybir as mybir
import concourse.tile as tile
from concourse.bass_utils import run_bass_kernel_spmd

# Problem shapes (hardcoded per contest contract)
B, S, IN, OUT = 4, 2048, 4096, 4096
E, R, K = 8, 64, 128
ER = E * R            # 512
SH = S // 2           # 1024 tokens per core
P = 128
NIO = IN // P         # 32 contraction subtiles
NN = OUT // 512       # 8 output column blocks
NM = SH // P          # 8 token row blocks
SCALE = 1.0           # (alpha/lora_dim) * multiplier
LSHIFT = 32.0         # host prescale of lora_down / lora_up (fp8 range)
TSCALE = 1.0 / (LSHIFT * LSHIFT)  # folded into the tT eviction

F32 = mybir.dt.float32
BF16 = mybir.dt.bfloat16
FP8 = mybir.dt.float8e4
BF16_NP = ml_dtypes.bfloat16
FP8_NP = ml_dtypes.float8_e4m3

_NC_CACHE = {}


def _build_nc(reps: int = 1):
    nc = bacc.Bacc("TRN2", target_bir_lowering=False, debug=False)

    xh = nc.dram_tensor("xh", [IN, SH], BF16, kind="ExternalInput")
    xh8 = nc.dram_tensor("xh8", [IN, SH], FP8, kind="ExternalInput")
    xo8 = nc.dram_tensor("xo8", [IN, SH], FP8, kind="ExternalInput")
    wT = nc.dram_tensor("wT", [IN, OUT], BF16, kind="ExternalInput")
    ldn8 = nc.dram_tensor("ldn8", [IN, ER], FP8, kind="ExternalInput")
    lupb = nc.dram_tensor("lupb", [ER, OUT], BF16, kind="ExternalInput")
    mk8 = nc.dram_tensor("mk8", [IN, 2 * E], FP8, kind="ExternalInput")
    cind = nc.dram_tensor("cind", [E, ER], F32, kind="ExternalInput")
    out = nc.dram_tensor("out", [SH, OUT], F32, kind="ExternalOutput")

    xh_ap, xh8_ap, xo8_ap, wT_ap, ldn8_ap, lupb_ap, mk8_ap, cind_ap, out_ap = (
        t.ap() for t in (xh, xh8, xo8, wT, ldn8, lupb, mk8, cind, out)
    )

    DR = mybir.MatmulPerfMode.DoubleRow
    trace_sim = os.environ.get("KERNEL_SIM_TRACE", "0") == "1"
    with tile.TileContext(nc, trace_sim=trace_sim) as tc:
        with (
            tc.tile_pool(name="xpool", bufs=1) as xpool,
            tc.tile_pool(name="x8pool", bufs=1) as x8pool,
            tc.tile_pool(name="xopool", bufs=2) as xopool,
            tc.tile_pool(name="spool", bufs=4) as spool,
            tc.tile_pool(name="tpool", bufs=1) as tpool,
            tc.tile_pool(name="lrpool", bufs=2) as lrpool,
            tc.tile_pool(name="lpool", bufs=2) as lpool,
            tc.tile_pool(name="opool", bufs=2) as opool,
            tc.tile_pool(name="rpool", bufs=1) as rpool,
            tc.tile_pool(name="ptp", bufs=2, space="PSUM") as ptp,
            tc.tile_pool(name="pop", bufs=4, space="PSUM") as pop,
            tc.tile_pool(name="prp", bufs=1, space="PSUM") as prp,
            tc.tile_pool(name="pccp", bufs=1, space="PSUM") as pccp,
        ):
            for rep in range(reps):
                _emit_once(
                    nc, rep,
                    xpool, x8pool, xopool, spool, tpool, lrpool, lpool,
                    opool, rpool, ptp, pop, prp, pccp,
                    xh_ap, xh8_ap, xo8_ap, wT_ap, ldn8_ap, lupb_ap, mk8_ap,
                    cind_ap, out_ap, DR,
                )

    nc.compile()
    return nc


def _emit_once(
    nc, rep,
    xpool, x8pool, xopool, spool, tpool, lrpool, lpool,
    opool, rpool, ptp, pop, prp, pccp,
    xh_ap, xh8_ap, xo8_ap, wT_ap, ldn8_ap, lupb_ap, mk8_ap, cind_ap, out_ap,
    DR,
):
    rn = lambda base: f"r{rep}_{base}"

    # --- persistent small tiles ---
    # E padded to 16 so the DoubleRow pair stride is 16B-aligned
    # (walrus s3_lw_dual_fp8_restrictions)
    mk8_sb = rpool.tile([P, NIO, 2 * E], FP8, tag="mk8_sb", name=rn("mk8_sb"))
    nc.sync.dma_start(mk8_sb[:], mk8_ap.rearrange("(io pp) e -> pp io e", pp=P))
    cind_sb = rpool.tile([E, ER], F32, tag="cind_sb", name=rn("cind_sb"))
    nc.sync.dma_start(cind_sb[:], cind_ap)
    ones8 = rpool.tile([E, 1], F32, tag="ones8", name=rn("ones8"))
    nc.any.memset(ones8[:], 1.0)
    ones_row = rpool.tile([1, P], F32, tag="ones_row", name=rn("ones_row"))
    nc.any.memset(ones_row[:], 1.0)
    csc8 = rpool.tile([E, 1], F32, tag="csc8", name=rn("csc8"))
    nc.any.memset(csc8[:], 1.0 / (S * math.sqrt(K)))
    ctsc = rpool.tile([P, 1], F32, tag="ctsc", name=rn("ctsc"))
    nc.any.memset(ctsc[:], TSCALE)

    tT = tpool.tile([P, 4, SH], FP8, tag="tT", name=rn("tT"))

    # --- input streams (balanced across the two DMA paths) ---
    xq8 = []
    for k in range(8):
        t = x8pool.tile([P, 4, SH], FP8, tag=f"xh8_{k}", name=rn(f"xh8_{k}"))
        eng = nc.sync if k % 2 == 0 else nc.gpsimd
        eng.dma_start(
            t[:],
            xh8_ap[k * 512 : (k + 1) * 512, :].rearrange(
                "(io pp) s -> pp io s", pp=P
            ),
        )
        xq8.append(t)

    ldcs = []
    for j in range(2):
        ldc = spool.tile([P, 16, ER], FP8, tag="stream", name=rn(f"ldc{j}"))
        (nc.sync if j == 0 else nc.gpsimd).dma_start(
            ldc[:],
            ldn8_ap[j * 2048 : (j + 1) * 2048, :].rearrange(
                "(io pp) e -> pp io e", pp=P
            ),
        )
        ldcs.append(ldc)

    xo8q = []
    for k in range(8):
        t = xopool.tile([P, 4, SH], FP8, tag="xo8", name=rn(f"xo8_{k}"))
        eng = nc.sync if k % 2 == 0 else nc.gpsimd
        eng.dma_start(
            t[:],
            xo8_ap[k * 512 : (k + 1) * 512, :].rearrange(
                "(io pp) s -> pp io s", pp=P
            ),
        )
        xo8q.append(t)

    def load_wc(n):
        wcs = []
        for j in range(2):
            wcj = spool.tile([P, 16, 512], BF16, tag="stream", name=rn(f"wc_{n}_{j}"))
            (nc.sync if j == 0 else nc.gpsimd).dma_start(
                wcj[:],
                wT_ap[
                    j * 2048 : (j + 1) * 2048, n * 512 : (n + 1) * 512
                ].rearrange("(io pp) o -> pp io o", pp=P),
            )
            wcs.append(wcj)
        return wcs

    wc_tiles = {0: load_wc(0)}

    xq = []
    for k in range(8):
        t = xpool.tile([P, 4, SH], BF16, tag=f"xh{k}", name=rn(f"xh{k}"))
        eng = nc.sync if k % 2 == 0 else nc.gpsimd
        eng.dma_start(
            t[:],
            xh_ap[k * 512 : (k + 1) * 512, :].rearrange(
                "(io pp) s -> pp io s", pp=P
            ),
        )
        xq.append(t)

    def xs(io, fslice):
        return xq[io // 4][:, io % 4, fslice]

    # fp8 x pair slice for DoubleRow: pair t covers io (2t, 2t+1)
    def x8pair(t, fslice):
        return xq8[t // 2][:, 2 * (t % 2) : 2 * (t % 2) + 2, fslice]

    # --- router projection on PE, fp8 DoubleRow, both halves, one group ---
    pr = prp.tile([E, 512], F32, tag="pr", name=rn("pr"))
    for cc in range(2):
        for t in range(16):
            nc.tensor.matmul(
                pr[:],
                mk8_sb[:, 2 * t : 2 * t + 2, 0:E],
                x8pair(t, slice(cc * 512, (cc + 1) * 512)),
                start=(cc == 0 and t == 0),
                stop=False,
                perf_mode=DR,
            )

    # --- T phase: tT[er, s] = (32*ldn).T @ x, fp8 DoubleRow ---
    for u in range(4):
        for cc in range(2):
            ps = ptp.tile([P, 512], F32, tag="pt", name=rn(f"pt_{u}_{cc}"))
            for t in range(16):
                nc.tensor.matmul(
                    ps[:],
                    ldcs[t // 8][:, 2 * (t % 8) : 2 * (t % 8) + 2, u * P : (u + 1) * P],
                    x8pair(t, slice(cc * 512, (cc + 1) * 512)),
                    start=(t == 0),
                    stop=(t == 15),
                    perf_mode=DR,
                )
            # rescale by 2^-10 (undoes the two host-side 32x prescales)
            nc.vector.tensor_scalar_mul(
                tT[:, u, cc * 512 : (cc + 1) * 512], ps[:], ctsc[:]
            )

    # --- router projection, other sequence half (streamed fp8) ---
    for k in range(8):
        for jj in range(2):
            for cc in range(2):
                nc.tensor.matmul(
                    pr[:],
                    mk8_sb[:, 4 * k + 2 * jj : 4 * k + 2 * jj + 2, 0:E],
                    xo8q[k][:, 2 * jj : 2 * jj + 2, cc * 512 : (cc + 1) * 512],
                    start=False,
                    stop=(k == 7 and jj == 1 and cc == 1),
                    perf_mode=DR,
                )

    # --- softmax + coefficient placement ---
    scores_raw = rpool.tile([E, 1], F32, tag="scores_raw", name=rn("scores_raw"))
    nc.vector.reduce_sum(scores_raw[:], pr[:], axis=mybir.AxisListType.X)
    scores = rpool.tile([E, 1], F32, tag="scores", name=rn("scores"))
    nc.vector.tensor_scalar_mul(scores[:], scores_raw[:], csc8[:])
    exps = rpool.tile([E, 1], F32, tag="exps", name=rn("exps"))
    nc.scalar.activation(exps[:], scores[:], mybir.ActivationFunctionType.Exp)
    psum_s = pccp.tile([1, 1], F32, tag="pcc", name=rn("psum_s"))
    nc.tensor.matmul(psum_s[:], exps[:], ones8[:], start=True, stop=True)
    rinv = rpool.tile([1, 1], F32, tag="rinv", name=rn("rinv"))
    nc.vector.reciprocal(rinv[:], psum_s[:])
    rb_p = pccp.tile([P, 1], F32, tag="pcc", name=rn("rb_p"))
    nc.tensor.matmul(rb_p[:], ones_row[:], rinv[:], start=True, stop=True)
    rb = rpool.tile([P, 1], F32, tag="rb", name=rn("rb"))
    nc.vector.tensor_copy(rb[:], rb_p[:])
    # cc_un[pp, u] = exp(score[(u*128+pp)//64])
    cc_un = rpool.tile([P, 4], F32, tag="cc_un", name=rn("cc_un"))
    for u in range(4):
        pcc = pccp.tile([P, 1], F32, tag="pcc", name=rn(f"pcc_{u}"))
        nc.tensor.matmul(
            pcc[:],
            cind_sb[:, u * P : (u + 1) * P],
            exps[:],
            start=True,
            stop=True,
        )
        nc.vector.tensor_copy(cc_un[:, u : u + 1], pcc[:])
    coeff_f = rpool.tile([P, 4], F32, tag="coeff_f", name=rn("coeff_f"))
    nc.vector.tensor_scalar_mul(coeff_f[:], cc_un[:], rb[:])
    coeff = rpool.tile([P, 4], BF16, tag="coeff", name=rn("coeff"))
    nc.vector.tensor_copy(coeff[:], coeff_f[:])

    # --- coefficient-scaled lora_up tiles (bf16 in, fp8 out) ---
    lsc_tiles = [None] * NN

    def load_lsc(n):
        lraw = lrpool.tile([P, 4, 512], BF16, tag="lraw", name=rn(f"lraw_{n}"))
        nc.gpsimd.dma_start(
            lraw[:],
            lupb_ap[:, n * 512 : (n + 1) * 512].rearrange(
                "(u pp) o -> pp u o", pp=P
            ),
        )
        t = lpool.tile([P, 4, 512], FP8, tag="lsc", name=rn(f"lsc_{n}"))
        nc.vector.tensor_tensor(
            t[:],
            lraw[:],
            coeff[:, :, None].to_broadcast((P, 4, 512)),
            mybir.AluOpType.mult,
        )
        lsc_tiles[n] = t

    load_lsc(0)

    # --- main loop: out = x @ W + tT.T @ lsc, fused in PSUM ---
    for n in range(NN):
        if n + 1 < NN:
            wc_tiles[n + 1] = load_wc(n + 1)
        wc = wc_tiles.pop(n)
        lsc = lsc_tiles[n]
        for m in range(NM):
            sl = (
                slice(m * P, (m + 1) * P),
                slice(n * 512, (n + 1) * 512),
            )
            po = pop.tile([P, 512], F32, tag="po", name=rn(f"po_{n}_{m}"))
            for io in range(NIO):
                nc.tensor.matmul(
                    po[:],
                    xs(io, slice(m * P, (m + 1) * P)),
                    wc[io // 16][:, io % 16, :],
                    start=(io == 0),
                    stop=False,
                )
            for t in range(2):
                nc.tensor.matmul(
                    po[:],
                    tT[:, 2 * t : 2 * t + 2, m * P : (m + 1) * P],
                    lsc[:, 2 * t : 2 * t + 2, :],
                    start=False,
                    stop=(t == 1),
                    perf_mode=DR,
                )
            ost = opool.tile([P, 512], F32, tag="ost", name=rn(f"ost_{n}_{m}"))
            nc.vector.tensor_copy(ost[:], po[:])
            (nc.sync if m % 2 == 0 else nc.gpsimd).dma_start(out_ap[sl], ost[:])
            if m == 0 and n + 1 < NN:
                # prefetch next block's scaled lora_up after the first
                # eviction so a late lraw DMA cannot stall the DVE queue
                # ahead of this block's evictions
                load_lsc(n + 1)


def kernel(x, W_org, lora_down, lora_up, lora_keys, Wq):
    x = np.asarray(x, dtype=np.float32)
    W_org = np.asarray(W_org, dtype=np.float32)
    lora_down = np.asarray(lora_down, dtype=np.float32)
    lora_up = np.asarray(lora_up, dtype=np.float32)
    lora_keys = np.asarray(lora_keys, dtype=np.float32)
    Wq = np.asarray(Wq, dtype=np.float32)

    # Host-side constant folding / layout prep (transposes to K-major)
    wT = np.ascontiguousarray(W_org.T).astype(BF16_NP)               # [IN, OUT]
    ldn8 = (
        np.ascontiguousarray(lora_down.transpose(1, 0, 2).reshape(IN, ER))
        * LSHIFT
    ).astype(FP8_NP)                                                 # [IN, ER]
    lupb = (np.ascontiguousarray(lora_up.reshape(ER, OUT)) * LSHIFT).astype(
        BF16_NP
    )
    mk8 = np.zeros((IN, 2 * E), dtype=FP8_NP)                        # [IN, 16]
    mk8[:, :E] = (Wq.T @ lora_keys.T).astype(FP8_NP)
    cind = np.repeat(np.eye(E, dtype=np.float32), R, axis=1)         # [E, ER]
    xT = [np.ascontiguousarray(x[b].T) for b in range(B)]            # [IN, S] f32

    if "nc" not in _NC_CACHE:
        _NC_CACHE["nc"] = _build_nc()
    nc = _NC_CACHE["nc"]

    in_maps = []
    for c in range(8):
        b, h = c // 2, c % 2
        own = xT[b][:, h * SH : (h + 1) * SH]
        other = xT[b][:, (1 - h) * SH : (2 - h) * SH]
        in_maps.append(
            {
                "xh": own.astype(BF16_NP),
                "xh8": own.astype(FP8_NP),
                "xo8": other.astype(FP8_NP),
                "wT": wT,
                "ldn8": ldn8,
                "lupb": lupb,
                "mk8": mk8,
                "cind": cind,
            }
        )

    res = run_bass_kernel_spmd(nc, in_maps, core_ids=list(range(8)), trace=False)
    _NC_CACHE["last_result"] = res
    _NC_CACHE["last_in_maps"] = in_maps

    outp = np.empty((B, S, OUT), dtype=np.float32)
    for c in range(8):
        b, h = c // 2, c % 2
        outp[b, h * SH : (h + 1) * SH, :] = res.results[c]["out"]
    return outp


def _dispatch_fn(nc, in_maps):
    """Build a jitted single-NEFF dispatcher (mirrors run_bass_via_pjrt's
    multi-core path with device-resident inputs)."""
    import jax
    from jax.experimental.shard_map import shard_map
    from jax.sharding import Mesh, NamedSharding, PartitionSpec

    from concourse import bass2jax, mybir as _mybir

    bass2jax.install_neuronx_cc_hook()

    n_cores = len(in_maps)
    partition_name = nc.partition_id_tensor.name if nc.partition_id_tensor else None
    in_names, out_names, out_avals, zero_outs = [], [], [], []
    for alloc in nc.m.functions[0].allocations:
        if not isinstance(alloc, _mybir.MemoryLocationSet):
            continue
        name = alloc.memorylocations[0].name
        if alloc.kind == "ExternalInput":
            if name != partition_name:
                in_names.append(name)
        elif alloc.kind == "ExternalOutput":
            aval = jax.core.ShapedArray(
                tuple(alloc.tensor_shape), _mybir.dt.np(alloc.dtype)
            )
            out_avals.append(aval)
            out_names.append(name)
            zero_outs.append(np.zeros(aval.shape, aval.dtype))
    n_params = len(in_names)
    n_outs = len(out_avals)
    all_in_names = in_names + out_names
    if partition_name is not None:
        all_in_names = all_in_names + [partition_name]

    def _body(*args):
        operands = list(args)
        if partition_name is not None:
            operands.append(bass2jax.partition_id_tensor())
        outs = bass2jax._bass_exec_p.bind(
            *operands,
            out_avals=tuple(out_avals),
            in_names=tuple(all_in_names),
            out_names=tuple(out_names),
            lowering_input_output_aliases=(),
            sim_require_finite=True,
            sim_require_nnan=True,
            nc=nc,
        )
        return tuple(outs)

    _body.__name__ = "_body"

    devices = jax.devices()[:n_cores]
    mesh = Mesh(np.asarray(devices), ("core",))
    spec = PartitionSpec("core")
    sharding = NamedSharding(mesh, spec)
    donate = tuple(range(n_params, n_params + n_outs))
    fn = jax.jit(
        shard_map(
            _body,
            mesh=mesh,
            in_specs=(spec,) * (n_params + n_outs),
            out_specs=(spec,) * n_outs,
            check_rep=False,
        ),
        donate_argnums=donate,
        keep_unused=True,
    )

    concat_in = [
        np.concatenate([np.asarray(in_maps[c][nm]) for c in range(n_cores)], axis=0)
        for nm in in_names
    ]
    concat_zero = [
        np.zeros((n_cores * z.shape[0], *z.shape[1:]), z.dtype) for z in zero_outs
    ]
    dev_in = [jax.device_put(a, sharding) for a in concat_in]
    for a in dev_in:
        a.block_until_ready()
    return fn, dev_in, concat_zero, sharding


def _time_dispatch(nc, in_maps, iters):
    import time
    import jax

    fn, dev_in, concat_zero, sharding = _dispatch_fn(nc, in_maps)
    # outputs of call k are recycled as the donated out-operands of call
    # k+1 (same shape/dtype/sharding), so nothing re-ships between trials
    outs = [jax.device_put(z, sharding) for z in concat_zero]
    for z in outs:
        z.block_until_ready()
    times = []
    for _ in range(iters + 1):
        t0 = time.perf_counter()
        outs = fn(*dev_in, *outs)
        for o in outs:
            o.block_until_ready()
        times.append(time.perf_counter() - t0)
    return times[1:]


def benchmark_chained(iters=12, reps_hi=16):
    """Estimate per-execution device time as
    (min t(NEFF with reps_hi bodies) - min t(NEFF with 1 body)) / (reps_hi-1).
    The repeated-body NEFF serializes reps on-device, so the delta is pure
    device time; the axon relay dispatch noise cancels in the min."""
    if "nc" not in _NC_CACHE:
        _NC_CACHE["nc"] = _build_nc()
    key = f"nc_reps{reps_hi}"
    if key not in _NC_CACHE:
        _NC_CACHE[key] = _build_nc(reps=reps_hi)
    in_maps = _NC_CACHE["last_in_maps"]
    t1 = _time_dispatch(_NC_CACHE["nc"], in_maps, iters)
    tN = _time_dispatch(_NC_CACHE[key], in_maps, iters)
    per_exec = (min(tN) - min(t1)) / (reps_hi - 1)
    return per_exec, {1: t1, reps_hi: tN}


def benchmark(iters: int = 8, nc=None):
    """Single-dispatch wall times (relay noise included)."""
    if nc is None:
        nc = _NC_CACHE["nc"]
    return _time_dispatch(nc, _NC_CACHE["last_in_maps"], iters)


# revision 10
# speedup vs baseline: 43.4911x; 1.0058x over previous
"""AttLoRA MoE-routing kernel for 8 Trainium2 NeuronCores.

Reference computation (per problem nn_AttLoRAModule_85839216378078):
    base  = x @ W_org.T                                    [B,S,OUT]
    q     = x.mean(axis=1) @ Wq.T                          [B,K]
    coef  = softmax(q @ lora_keys.T / sqrt(K))             [B,E]
    h     = x @ lora_down[e]                               [B,S,E,R]
    delta = sum_e coef[b,e] * (h[...,e,:] @ lora_up[e])    [B,S,OUT]
    out   = base + delta * SCALE        (SCALE = 1.0 here)

Sharding: 8 cores = 4 batches x 2 sequence-halves.  Core c handles batch
c//2, tokens [(c%2)*1024, (c%2+1)*1024), all OUT columns.  The sequence
split (instead of an OUT split) halves the per-core x@lora_down work.

Device strategy (per core):
  - base GEMM in bf16 (1 PE cycle/row, = f32r rate, half the SBUF/DMA):
    x own-half [4096,1024] bf16 SBUF-resident; 64 PSUM groups of
    32 base matmuls + 2 fp8-DoubleRow delta matmuls; single eviction.
  - LoRA T-phase (x @ lora_down) and delta (tT.T @ lsc) in fp8e4m3 with
    perf_mode=DoubleRow (2 contraction rows/cycle): 131k -> 33k PE
    cycles each.  Host prescales lora_down/lora_up by 32 to dodge fp8
    subnormals; the tT eviction rescales by 2^-10.
  - Router entirely on PE in fp8 DoubleRow: pr[e,j] accumulates
    x[:,s] @ (Wq.T@keys.T) over BOTH sequence halves (other half
    streamed as fp8), then reduce + softmax on one partition and
    coefficients broadcast via indicator (cind) matmuls.  Coefficients
    are folded into bf16 lora_up tiles -> fp8 lsc.
  - PE work/core ~= 1.12M cycles ~= 468 us at warm 2.4 GHz.

The `reps` build parameter emits the whole computation N times in one
NEFF (tile tags shared across reps so SBUF slots rotate) — used only for
benchmarking: per-exec time = (t(reps_hi) - t(1)) / (reps_hi - 1), which
cancels the tens-of-ms axon relay dispatch noise.
"""

import math
import os

import numpy as np
import ml_dtypes

import concourse.bacc as bacc
import concourse.mybir as mybir
import concourse.tile as tile
from concourse.bass_utils import run_bass_kernel_spmd

# Problem shapes (hardcoded per contest contract)
B, S, IN, OUT = 4, 2048, 4096, 4096
E, R, K = 8, 64, 128
ER = E * R            # 512
SH = S // 2           # 1024 tokens per core
P = 128
NIO = IN // P         # 32 contraction subtiles
NN = OUT // 512       # 8 output column blocks
NM = SH // P          # 8 token row blocks
SCALE = 1.0           # (alpha/lora_dim) * multiplier
LSHIFT = 32.0         # host prescale of lora_down / lora_up (fp8 range)
TSCALE = 1.0 / (LSHIFT * LSHIFT)  # folded into the tT eviction

F32 = mybir.dt.float32
BF16 = mybir.dt.bfloat16
FP8 = mybir.dt.float8e4
BF16_NP = ml_dtypes.bfloat16
FP8_NP = ml_dtypes.float8_e4m3

_NC_CACHE = {}


def _build_nc(reps: int = 1):
    nc = bacc.Bacc("TRN2", target_bir_lowering=False, debug=False)

    xh = nc.dram_tensor("xh", [IN, SH], BF16, kind="ExternalInput")
    xh8 = nc.dram_tensor("xh8", [IN, SH], FP8, kind="ExternalInput")
    xo8 = nc.dram_tensor("xo8", [IN, SH], FP8, kind="ExternalInput")
    wT = nc.dram_tensor("wT", [IN, OUT], BF16, kind="ExternalInput")
    ldn8 = nc.dram_tensor("ldn8", [IN, ER], FP8, kind="ExternalInput")
    lupb = nc.dram_tensor("lupb", [ER, OUT], BF16, kind="ExternalInput")
    mk8 = nc.dram_tensor("mk8", [IN, 2 * E], FP8, kind="ExternalInput")
    cind = nc.dram_tensor("cind", [E, ER], F32, kind="ExternalInput")
    out = nc.dram_tensor("out", [SH, OUT], F32, kind="ExternalOutput")

    xh_ap, xh8_ap, xo8_ap, wT_ap, ldn8_ap, lupb_ap, mk8_ap, cind_ap, out_ap = (
        t.ap() for t in (xh, xh8, xo8, wT, ldn8, lupb, mk8, cind, out)
    )

    DR = mybir.MatmulPerfMode.DoubleRow
    trace_sim = os.environ.get("KERNEL_SIM_TRACE", "0") == "1"
    with tile.TileContext(nc, trace_sim=trace_sim) as tc:
        with (
            tc.tile_pool(name="xpool", bufs=1) as xpool,
            tc.tile_pool(name="x8pool", bufs=1) as x8pool,
            tc.tile_pool(name="xopool", bufs=2) as xopool,
            tc.tile_pool(name="spool", bufs=4) as spool,
            tc.tile_pool(name="tpool", bufs=1) as tpool,
            tc.tile_pool(name="lrpool", bufs=2) as lrpool,
            tc.tile_pool(name="lpool", bufs=2) as lpool,
            tc.tile_pool(name="opool", bufs=2) as opool,
            tc.tile_pool(name="rpool", bufs=1) as rpool,
            tc.tile_pool(name="ptp", bufs=2, space="PSUM") as ptp,
            tc.tile_pool(name="pop", bufs=4, space="PSUM") as pop,
            tc.tile_pool(name="prp", bufs=1, space="PSUM") as prp,
            tc.tile_pool(name="pccp", bufs=1, space="PSUM") as pccp,
        ):
            for rep in range(reps):
                _emit_once(
                    nc, rep,
                    xpool, x8pool, xopool, spool, tpool, lrpool, lpool,
                    opool, rpool, ptp, pop, prp, pccp,
                    xh_ap, xh8_ap, xo8_ap, wT_ap, ldn8_ap, lupb_ap, mk8_ap,
                    cind_ap, out_ap, DR,
                )

    nc.compile()
    return nc


def _emit_once(
    nc, rep,
    xpool, x8pool, xopool, spool, tpool, lrpool, lpool,
    opool, rpool, ptp, pop, prp, pccp,
    xh_ap, xh8_ap, xo8_ap, wT_ap, ldn8_ap, lupb_ap, mk8_ap, cind_ap, out_ap,
    DR,
):
    rn = lambda base: f"r{rep}_{base}"

    # --- persistent small tiles ---
    # E padded to 16 so the DoubleRow pair stride is 16B-aligned
    # (walrus s3_lw_dual_fp8_restrictions)
    mk8_sb = rpool.tile([P, NIO, 2 * E], FP8, tag="mk8_sb", name=rn("mk8_sb"))
    nc.sync.dma_start(mk8_sb[:], mk8_ap.rearrange("(io pp) e -> pp io e", pp=P))
    cind_sb = rpool.tile([E, ER], F32, tag="cind_sb", name=rn("cind_sb"))
    nc.sync.dma_start(cind_sb[:], cind_ap)
    ones8 = rpool.tile([E, 1], F32, tag="ones8", name=rn("ones8"))
    nc.any.memset(ones8[:], 1.0)
    ones_row = rpool.tile([1, P], F32, tag="ones_row", name=rn("ones_row"))
    nc.any.memset(ones_row[:], 1.0)
    csc8 = rpool.tile([E, 1], F32, tag="csc8", name=rn("csc8"))
    nc.any.memset(csc8[:], 1.0 / (S * math.sqrt(K)))
    ctsc = rpool.tile([P, 1], F32, tag="ctsc", name=rn("ctsc"))
    nc.any.memset(ctsc[:], TSCALE)

    tT = tpool.tile([P, 4, SH], FP8, tag="tT", name=rn("tT"))

    # --- input streams (balanced across the two DMA paths) ---
    xq8 = []
    for k in range(8):
        t = x8pool.tile([P, 4, SH], FP8, tag=f"xh8_{k}", name=rn(f"xh8_{k}"))
        eng = nc.sync if k % 2 == 0 else nc.gpsimd
        eng.dma_start(
            t[:],
            xh8_ap[k * 512 : (k + 1) * 512, :].rearrange(
                "(io pp) s -> pp io s", pp=P
            ),
        )
        xq8.append(t)

    ldcs = []
    for j in range(2):
        ldc = spool.tile([P, 16, ER], FP8, tag="stream", name=rn(f"ldc{j}"))
        nc.scalar.dma_start(
            ldc[:],
            ldn8_ap[j * 2048 : (j + 1) * 2048, :].rearrange(
                "(io pp) e -> pp io e", pp=P
            ),
        )
        ldcs.append(ldc)

    xo8q = []
    for k in range(8):
        t = xopool.tile([P, 4, SH], FP8, tag="xo8", name=rn(f"xo8_{k}"))
        eng = nc.scalar
        eng.dma_start(
            t[:],
            xo8_ap[k * 512 : (k + 1) * 512, :].rearrange(
                "(io pp) s -> pp io s", pp=P
            ),
        )
        xo8q.append(t)

    def load_wc(n):
        wcs = []
        for j in range(2):
            wcj = spool.tile([P, 16, 512], BF16, tag="stream", name=rn(f"wc_{n}_{j}"))
            (nc.sync if j == 0 else nc.gpsimd).dma_start(
                wcj[:],
                wT_ap[
                    j * 2048 : (j + 1) * 2048, n * 512 : (n + 1) * 512
                ].rearrange("(io pp) o -> pp io o", pp=P),
            )
            wcs.append(wcj)
        return wcs

    wc_tiles = {0: load_wc(0)}

    xq = []
    for k in range(8):
        t = xpool.tile([P, 4, SH], BF16, tag=f"xh{k}", name=rn(f"xh{k}"))
        eng = nc.sync if k % 2 == 0 else nc.gpsimd
        eng.dma_start(
            t[:],
            xh_ap[k * 512 : (k + 1) * 512, :].rearrange(
                "(io pp) s -> pp io s", pp=P
            ),
        )
        xq.append(t)

    def xs(io, fslice):
        return xq[io // 4][:, io % 4, fslice]

    # fp8 x pair slice for DoubleRow: pair t covers io (2t, 2t+1)
    def x8pair(t, fslice):
        return xq8[t // 2][:, 2 * (t % 2) : 2 * (t % 2) + 2, fslice]

    # --- router projection on PE, fp8 DoubleRow, both halves, one group ---
    pr = prp.tile([E, 512], F32, tag="pr", name=rn("pr"))
    for cc in range(2):
        for t in range(16):
            nc.tensor.matmul(
                pr[:],
                mk8_sb[:, 2 * t : 2 * t + 2, 0:E],
                x8pair(t, slice(cc * 512, (cc + 1) * 512)),
                start=(cc == 0 and t == 0),
                stop=False,
                perf_mode=DR,
            )

    # --- T phase: tT[er, s] = (32*ldn).T @ x, fp8 DoubleRow ---
    for u in range(4):
        for cc in range(2):
            ps = ptp.tile([P, 512], F32, tag="pt", name=rn(f"pt_{u}_{cc}"))
            for t in range(16):
                nc.tensor.matmul(
                    ps[:],
                    ldcs[t // 8][:, 2 * (t % 8) : 2 * (t % 8) + 2, u * P : (u + 1) * P],
                    x8pair(t, slice(cc * 512, (cc + 1) * 512)),
                    start=(t == 0),
                    stop=(t == 15),
                    perf_mode=DR,
                )
            # rescale by 2^-10 (undoes the two host-side 32x prescales)
            nc.vector.tensor_scalar_mul(
                tT[:, u, cc * 512 : (cc + 1) * 512], ps[:], ctsc[:]
            )

    # --- router projection, other sequence half (streamed fp8) ---
    for k in range(8):
        for jj in range(2):
            for cc in range(2):
                nc.tensor.matmul(
                    pr[:],
                    mk8_sb[:, 4 * k + 2 * jj : 4 * k + 2 * jj + 2, 0:E],
                    xo8q[k][:, 2 * jj : 2 * jj + 2, cc * 512 : (cc + 1) * 512],
                    start=False,
                    stop=(k == 7 and jj == 1 and cc == 1),
                    perf_mode=DR,
                )

    # --- softmax + coefficient placement ---
    scores_raw = rpool.tile([E, 1], F32, tag="scores_raw", name=rn("scores_raw"))
    nc.vector.reduce_sum(scores_raw[:], pr[:], axis=mybir.AxisListType.X)
    scores = rpool.tile([E, 1], F32, tag="scores", name=rn("scores"))
    nc.vector.tensor_scalar_mul(scores[:], scores_raw[:], csc8[:])
    exps = rpool.tile([E, 1], F32, tag="exps", name=rn("exps"))
    nc.scalar.activation(exps[:], scores[:], mybir.ActivationFunctionType.Exp)
    psum_s = pccp.tile([1, 1], F32, tag="pcc", name=rn("psum_s"))
    nc.tensor.matmul(psum_s[:], exps[:], ones8[:], start=True, stop=True)
    rinv = rpool.tile([1, 1], F32, tag="rinv", name=rn("rinv"))
    nc.vector.reciprocal(rinv[:], psum_s[:])
    rb_p = pccp.tile([P, 1], F32, tag="pcc", name=rn("rb_p"))
    nc.tensor.matmul(rb_p[:], ones_row[:], rinv[:], start=True, stop=True)
    rb = rpool.tile([P, 1], F32, tag="rb", name=rn("rb"))
    nc.vector.tensor_copy(rb[:], rb_p[:])
    # cc_un[pp, u] = exp(score[(u*128+pp)//64])
    cc_un = rpool.tile([P, 4], F32, tag="cc_un", name=rn("cc_un"))
    for u in range(4):
        pcc = pccp.tile([P, 1], F32, tag="pcc", name=rn(f"pcc_{u}"))
        nc.tensor.matmul(
            pcc[:],
            cind_sb[:, u * P : (u + 1) * P],
            exps[:],
            start=True,
            stop=True,
        )
        nc.vector.tensor_copy(cc_un[:, u : u + 1], pcc[:])
    coeff_f = rpool.tile([P, 4], F32, tag="coeff_f", name=rn("coeff_f"))
    nc.vector.tensor_scalar_mul(coeff_f[:], cc_un[:], rb[:])
    coeff = rpool.tile([P, 4], BF16, tag="coeff", name=rn("coeff"))
    nc.vector.tensor_copy(coeff[:], coeff_f[:])

    # --- coefficient-scaled lora_up tiles (bf16 in, fp8 out) ---
    lsc_tiles = [None] * NN

    def load_lsc(n):
        lraw = lrpool.tile([P, 4, 512], BF16, tag="lraw", name=rn(f"lraw_{n}"))
        nc.gpsimd.dma_start(
            lraw[:],
            lupb_ap[:, n * 512 : (n + 1) * 512].rearrange(
                "(u pp) o -> pp u o", pp=P
            ),
        )
        t = lpool.tile([P, 4, 512], FP8, tag="lsc", name=rn(f"lsc_{n}"))
        nc.vector.tensor_tensor(
            t[:],
            lraw[:],
            coeff[:, :, None].to_broadcast((P, 4, 512)),
            mybir.AluOpType.mult,
        )
        lsc_tiles[n] = t

    load_lsc(0)

    # --- main loop: out = x @ W + tT.T @ lsc, fused in PSUM ---
    for n in range(NN):
        if n + 1 < NN:
            wc_tiles[n + 1] = load_wc(n + 1)
        wc = wc_tiles.pop(n)
        lsc = lsc_tiles[n]
        for m in range(NM):
            sl = (
                slice(m * P, (m + 1) * P),
                slice(n * 512, (n + 1) * 512),
            )
            po = pop.tile([P, 512], F32, tag="po", name=rn(f"po_{n}_{m}"))
            for io in range(NIO):
                nc.tensor.matmul(
                    po[:],
                    xs(io, slice(m * P, (m + 1) * P)),
                    wc[io // 16][:, io % 16, :],
                    start=(io == 0),
                    stop=False,
                )
            for t in range(2):
                nc.tensor.matmul(
                    po[:],
                    tT[:, 2 * t : 2 * t + 2, m * P : (m + 1) * P],
                    lsc[:, 2 * t : 2 * t + 2, :],
                    start=False,
                    stop=(t == 1),
                    perf_mode=DR,
                )
            ost = opool.tile([P, 512], F32, tag="ost", name=rn(f"ost_{n}_{m}"))
            nc.vector.tensor_copy(ost[:], po[:])
            (nc.sync if m % 2 == 0 else nc.gpsimd).dma_start(out_ap[sl], ost[:])
            if m == 0 and n + 1 < NN:
                # prefetch next block's scaled lora_up after the first
                # eviction so a late lraw DMA cannot stall the DVE queue
                # ahead of this block's evictions
                load_lsc(n + 1)


def kernel(x, W_org, lora_down, lora_up, lora_keys, Wq):
    x = np.asarray(x, dtype=np.float32)
    W_org = np.asarray(W_org, dtype=np.float32)
    lora_down = np.asarray(lora_down, dtype=np.float32)
    lora_up = np.asarray(lora_up, dtype=np.float32)
    lora_keys = np.asarray(lora_keys, dtype=np.float32)
    Wq = np.asarray(Wq, dtype=np.float32)

    # Host-side constant folding / layout prep (transposes to K-major)
    wT = np.ascontiguousarray(W_org.T).astype(BF16_NP)               # [IN, OUT]
    ldn8 = (
        np.ascontiguousarray(lora_down.transpose(1, 0, 2).reshape(IN, ER))
        * LSHIFT
    ).astype(FP8_NP)                                                 # [IN, ER]
    lupb = (np.ascontiguousarray(lora_up.reshape(ER, OUT)) * LSHIFT).astype(
        BF16_NP
    )
    mk8 = np.zeros((IN, 2 * E), dtype=FP8_NP)                        # [IN, 16]
    mk8[:, :E] = (Wq.T @ lora_keys.T).astype(FP8_NP)
    cind = np.repeat(np.eye(E, dtype=np.float32), R, axis=1)         # [E, ER]
    xT = [np.ascontiguousarray(x[b].T) for b in range(B)]            # [IN, S] f32

    if "nc" not in _NC_CACHE:
        _NC_CACHE["nc"] = _build_nc()
    nc = _NC_CACHE["nc"]

    in_maps = []
    for c in range(8):
        b, h = c // 2, c % 2
        own = xT[b][:, h * SH : (h + 1) * SH]
        other = xT[b][:, (1 - h) * SH : (2 - h) * SH]
        in_maps.append(
            {
                "xh": own.astype(BF16_NP),
                "xh8": own.astype(FP8_NP),
                "xo8": other.astype(FP8_NP),
                "wT": wT,
                "ldn8": ldn8,
                "lupb": lupb,
                "mk8": mk8,
                "cind": cind,
            }
        )

    res = run_bass_kernel_spmd(nc, in_maps, core_ids=list(range(8)), trace=False)
    _NC_CACHE["last_result"] = res
    _NC_CACHE["last_in_maps"] = in_maps

    outp = np.empty((B, S, OUT), dtype=np.float32)
    for c in range(8):
        b, h = c // 2, c % 2
        outp[b, h * SH : (h + 1) * SH, :] = res.results[c]["out"]
    return outp


def _dispatch_fn(nc, in_maps):
    """Build a jitted single-NEFF dispatcher (mirrors run_bass_via_pjrt's
    multi-core path with device-resident inputs)."""
    import jax
    from jax.experimental.shard_map import shard_map
    from jax.sharding import Mesh, NamedSharding, PartitionSpec

    from concourse import bass2jax, mybir as _mybir

    bass2jax.install_neuronx_cc_hook()

    n_cores = len(in_maps)
    partition_name = nc.partition_id_tensor.name if nc.partition_id_tensor else None
    in_names, out_names, out_avals, zero_outs = [], [], [], []
    for alloc in nc.m.functions[0].allocations:
        if not isinstance(alloc, _mybir.MemoryLocationSet):
            continue
        name = alloc.memorylocations[0].name
        if alloc.kind == "ExternalInput":
            if name != partition_name:
                in_names.append(name)
        elif alloc.kind == "ExternalOutput":
            aval = jax.core.ShapedArray(
                tuple(alloc.tensor_shape), _mybir.dt.np(alloc.dtype)
            )
            out_avals.append(aval)
            out_names.append(name)
            zero_outs.append(np.zeros(aval.shape, aval.dtype))
    n_params = len(in_names)
    n_outs = len(out_avals)
    all_in_names = in_names + out_names
    if partition_name is not None:
        all_in_names = all_in_names + [partition_name]

    def _body(*args):
        operands = list(args)
        if partition_name is not None:
            operands.append(bass2jax.partition_id_tensor())
        outs = bass2jax._bass_exec_p.bind(
            *operands,
            out_avals=tuple(out_avals),
            in_names=tuple(all_in_names),
            out_names=tuple(out_names),
            lowering_input_output_aliases=(),
            sim_require_finite=True,
            sim_require_nnan=True,
            nc=nc,
        )
        return tuple(outs)

    _body.__name__ = "_body"

    devices = jax.devices()[:n_cores]
    mesh = Mesh(np.asarray(devices), ("core",))
    spec = PartitionSpec("core")
    sharding = NamedSharding(mesh, spec)
    donate = tuple(range(n_params, n_params + n_outs))
    fn = jax.jit(
        shard_map(
            _body,
            mesh=mesh,
            in_specs=(spec,) * (n_params + n_outs),
            out_specs=(spec,) * n_outs,
            check_rep=False,
        ),
        donate_argnums=donate,
        keep_unused=True,
    )

    concat_in = [
        np.concatenate([np.asarray(in_maps[c][nm]) for c in range(n_cores)], axis=0)
        for nm in in_names
    ]
    concat_zero = [
        np.zeros((n_cores * z.shape[0], *z.shape[1:]), z.dtype) for z in zero_outs
    ]
    dev_in = [jax.device_put(a, sharding) for a in concat_in]
    for a in dev_in:
        a.block_until_ready()
    return fn, dev_in, concat_zero, sharding


def _time_dispatch(nc, in_maps, iters):
    import time
    import jax

    fn, dev_in, concat_zero, sharding = _dispatch_fn(nc, in_maps)
    # outputs of call k are recycled as the donated out-operands of call
    # k+1 (same shape/dtype/sharding), so nothing re-ships between trials
    outs = [jax.device_put(z, sharding) for z in concat_zero]
    for z in outs:
        z.block_until_ready()
    times = []
    for _ in range(iters + 1):
        t0 = time.perf_counter()
        outs = fn(*dev_in, *outs)
        for o in outs:
            o.block_until_ready()
        times.append(time.perf_counter() - t0)
    return times[1:]


def benchmark_chained(iters=12, reps_hi=16):
    """Estimate per-execution device time as
    (min t(NEFF with reps_hi bodies) - min t(NEFF with 1 body)) / (reps_hi-1).
    The repeated-body NEFF serializes reps on-device, so the delta is pure
    device time; the axon relay dispatch noise cancels in the min."""
    if "nc" not in _NC_CACHE:
        _NC_CACHE["nc"] = _build_nc()
    key = f"nc_reps{reps_hi}"
    if key not in _NC_CACHE:
        _NC_CACHE[key] = _build_nc(reps=reps_hi)
    in_maps = _NC_CACHE["last_in_maps"]
    t1 = _time_dispatch(_NC_CACHE["nc"], in_maps, iters)
    tN = _time_dispatch(_NC_CACHE[key], in_maps, iters)
    per_exec = (min(tN) - min(t1)) / (reps_hi - 1)
    return per_exec, {1: t1, reps_hi: tN}


def benchmark(iters: int = 8, nc=None):
    """Single-dispatch wall times (relay noise included)."""
    if nc is None:
        nc = _NC_CACHE["nc"]
    return _time_dispatch(nc, _NC_CACHE["last_in_maps"], iters)


# revision 12
# speedup vs baseline: 43.6584x; 1.0038x over previous
"""AttLoRA MoE-routing kernel for 8 Trainium2 NeuronCores.

Reference computation (per problem nn_AttLoRAModule_85839216378078):
    base  = x @ W_org.T                                    [B,S,OUT]
    q     = x.mean(axis=1) @ Wq.T                          [B,K]
    coef  = softmax(q @ lora_keys.T / sqrt(K))             [B,E]
    h     = x @ lora_down[e]                               [B,S,E,R]
    delta = sum_e coef[b,e] * (h[...,e,:] @ lora_up[e])    [B,S,OUT]
    out   = base + delta * SCALE        (SCALE = 1.0 here)

Sharding: 8 cores = 4 batches x 2 sequence-halves.  Core c handles batch
c//2, tokens [(c%2)*1024, (c%2+1)*1024), all OUT columns.  The sequence
split (instead of an OUT split) halves the per-core x@lora_down work.

Device strategy (per core):
  - base GEMM in bf16 (1 PE cycle/row, = f32r rate, half the SBUF/DMA):
    x own-half [4096,1024] bf16 SBUF-resident; 64 PSUM groups of
    32 base matmuls + 2 fp8-DoubleRow delta matmuls; single eviction.
  - LoRA T-phase (x @ lora_down) and delta (tT.T @ lsc) in fp8e4m3 with
    perf_mode=DoubleRow (2 contraction rows/cycle): 131k -> 33k PE
    cycles each.  Host prescales lora_down/lora_up by 32 to dodge fp8
    subnormals; the tT eviction rescales by 2^-10.
  - Router entirely on PE in fp8 DoubleRow: pr[e,j] accumulates
    x[:,s] @ (Wq.T@keys.T) over BOTH sequence halves (other half
    streamed as fp8), then reduce + softmax on one partition and
    coefficients broadcast via indicator (cind) matmuls.  Coefficients
    are folded into bf16 lora_up tiles -> fp8 lsc.
  - PE work/core ~= 1.12M cycles ~= 468 us at warm 2.4 GHz.

The `reps` build parameter emits the whole computation N times in one
NEFF (tile tags shared across reps so SBUF slots rotate) — used only for
benchmarking: per-exec time = (t(reps_hi) - t(1)) / (reps_hi - 1), which
cancels the tens-of-ms axon relay dispatch noise.
"""

import math
import os

import numpy as np
import ml_dtypes

import concourse.bacc as bacc
import concourse.mybir as mybir
import concourse.tile as tile
from concourse.bass_utils import run_bass_kernel_spmd

# Problem shapes (hardcoded per contest contract)
B, S, IN, OUT = 4, 2048, 4096, 4096
E, R, K = 8, 64, 128
ER = E * R            # 512
SH = S // 2           # 1024 tokens per core
P = 128
NIO = IN // P         # 32 contraction subtiles
NN = OUT // 512       # 8 output column blocks
NM = SH // P          # 8 token row blocks
SCALE = 1.0           # (alpha/lora_dim) * multiplier
LSHIFT = 32.0         # host prescale of lora_down / lora_up (fp8 range)
TSCALE = 1.0 / (LSHIFT * LSHIFT)  # folded into the tT eviction

F32 = mybir.dt.float32
BF16 = mybir.dt.bfloat16
FP8 = mybir.dt.float8e4
BF16_NP = ml_dtypes.bfloat16
FP8_NP = ml_dtypes.float8_e4m3

_NC_CACHE = {}


def _build_nc(reps: int = 1):
    nc = bacc.Bacc("TRN2", target_bir_lowering=False, debug=False)

    xh = nc.dram_tensor("xh", [IN, SH], BF16, kind="ExternalInput")
    xh8 = nc.dram_tensor("xh8", [IN, SH], FP8, kind="ExternalInput")
    xo8 = nc.dram_tensor("xo8", [IN, SH], FP8, kind="ExternalInput")
    wT = nc.dram_tensor("wT", [IN, OUT], BF16, kind="ExternalInput")
    ldn8 = nc.dram_tensor("ldn8", [IN, ER], FP8, kind="ExternalInput")
    lupb = nc.dram_tensor("lupb", [ER, OUT], BF16, kind="ExternalInput")
    mk8 = nc.dram_tensor("mk8", [IN, 2 * E], FP8, kind="ExternalInput")
    cind = nc.dram_tensor("cind", [E, ER], F32, kind="ExternalInput")
    out = nc.dram_tensor("out", [SH, OUT], F32, kind="ExternalOutput")

    xh_ap, xh8_ap, xo8_ap, wT_ap, ldn8_ap, lupb_ap, mk8_ap, cind_ap, out_ap = (
        t.ap() for t in (xh, xh8, xo8, wT, ldn8, lupb, mk8, cind, out)
    )

    DR = mybir.MatmulPerfMode.DoubleRow
    trace_sim = os.environ.get("KERNEL_SIM_TRACE", "0") == "1"
    with tile.TileContext(nc, trace_sim=trace_sim) as tc:
        with (
            tc.tile_pool(name="xpool", bufs=1) as xpool,
            tc.tile_pool(name="x8pool", bufs=1) as x8pool,
            tc.tile_pool(name="xopool", bufs=2) as xopool,
            tc.tile_pool(name="spool", bufs=4) as spool,
            tc.tile_pool(name="tpool", bufs=1) as tpool,
            tc.tile_pool(name="lrpool", bufs=2) as lrpool,
            tc.tile_pool(name="lpool", bufs=2) as lpool,
            tc.tile_pool(name="opool", bufs=2) as opool,
            tc.tile_pool(name="rpool", bufs=1) as rpool,
            tc.tile_pool(name="ptp", bufs=2, space="PSUM") as ptp,
            tc.tile_pool(name="pop", bufs=4, space="PSUM") as pop,
            tc.tile_pool(name="prp", bufs=1, space="PSUM") as prp,
            tc.tile_pool(name="pccp", bufs=1, space="PSUM") as pccp,
        ):
            for rep in range(reps):
                _emit_once(
                    nc, rep,
                    xpool, x8pool, xopool, spool, tpool, lrpool, lpool,
                    opool, rpool, ptp, pop, prp, pccp,
                    xh_ap, xh8_ap, xo8_ap, wT_ap, ldn8_ap, lupb_ap, mk8_ap,
                    cind_ap, out_ap, DR,
                )

    nc.compile()
    return nc


def _emit_once(
    nc, rep,
    xpool, x8pool, xopool, spool, tpool, lrpool, lpool,
    opool, rpool, ptp, pop, prp, pccp,
    xh_ap, xh8_ap, xo8_ap, wT_ap, ldn8_ap, lupb_ap, mk8_ap, cind_ap, out_ap,
    DR,
):
    rn = lambda base: f"r{rep}_{base}"

    # --- persistent small tiles ---
    # E padded to 16 so the DoubleRow pair stride is 16B-aligned
    # (walrus s3_lw_dual_fp8_restrictions)
    mk8_sb = rpool.tile([P, NIO, 2 * E], FP8, tag="mk8_sb", name=rn("mk8_sb"))
    nc.sync.dma_start(mk8_sb[:], mk8_ap.rearrange("(io pp) e -> pp io e", pp=P))
    cind_sb = rpool.tile([E, ER], F32, tag="cind_sb", name=rn("cind_sb"))
    nc.scalar.dma_start(cind_sb[:], cind_ap)
    ones8 = rpool.tile([E, 1], F32, tag="ones8", name=rn("ones8"))
    nc.any.memset(ones8[:], 1.0)
    ones_row = rpool.tile([1, P], F32, tag="ones_row", name=rn("ones_row"))
    nc.any.memset(ones_row[:], 1.0)
    csc8 = rpool.tile([E, 1], F32, tag="csc8", name=rn("csc8"))
    nc.any.memset(csc8[:], 1.0 / (S * math.sqrt(K)))
    ctsc = rpool.tile([P, 1], F32, tag="ctsc", name=rn("ctsc"))
    nc.any.memset(ctsc[:], TSCALE)

    tT = tpool.tile([P, 4, SH], FP8, tag="tT", name=rn("tT"))

    # --- input streams (balanced across the two DMA paths) ---
    xq8 = []
    for k in range(16):
        t = x8pool.tile([P, 2, SH], FP8, tag=f"xh8_{k}", name=rn(f"xh8_{k}"))
        eng = nc.sync if k % 2 == 0 else nc.gpsimd
        eng.dma_start(
            t[:],
            xh8_ap[k * 256 : (k + 1) * 256, :].rearrange(
                "(io pp) s -> pp io s", pp=P
            ),
        )
        xq8.append(t)

    ldcs = []
    for j in range(2):
        ldc = spool.tile([P, 16, ER], FP8, tag="stream", name=rn(f"ldc{j}"))
        nc.scalar.dma_start(
            ldc[:],
            ldn8_ap[j * 2048 : (j + 1) * 2048, :].rearrange(
                "(io pp) e -> pp io e", pp=P
            ),
        )
        ldcs.append(ldc)

    xo8q = []
    for k in range(8):
        t = xopool.tile([P, 4, SH], FP8, tag="xo8", name=rn(f"xo8_{k}"))
        eng = nc.scalar
        eng.dma_start(
            t[:],
            xo8_ap[k * 512 : (k + 1) * 512, :].rearrange(
                "(io pp) s -> pp io s", pp=P
            ),
        )
        xo8q.append(t)

    def load_wc(n):
        wcs = []
        for j in range(2):
            wcj = spool.tile([P, 16, 512], BF16, tag="stream", name=rn(f"wc_{n}_{j}"))
            (nc.sync if j == 0 else nc.gpsimd).dma_start(
                wcj[:],
                wT_ap[
                    j * 2048 : (j + 1) * 2048, n * 512 : (n + 1) * 512
                ].rearrange("(io pp) o -> pp io o", pp=P),
            )
            wcs.append(wcj)
        return wcs

    wc_tiles = {0: load_wc(0)}

    xq = []
    for k in range(8):
        t = xpool.tile([P, 4, SH], BF16, tag=f"xh{k}", name=rn(f"xh{k}"))
        eng = nc.sync if k % 2 == 0 else nc.gpsimd
        eng.dma_start(
            t[:],
            xh_ap[k * 512 : (k + 1) * 512, :].rearrange(
                "(io pp) s -> pp io s", pp=P
            ),
        )
        xq.append(t)

    def xs(io, fslice):
        return xq[io // 4][:, io % 4, fslice]

    # fp8 x pair slice for DoubleRow: tile t holds exactly io pair (2t, 2t+1)
    def x8pair(t, fslice):
        return xq8[t][:, :, fslice]

    # --- router projection on PE, fp8 DoubleRow, both halves, one group ---
    pr = prp.tile([E, 512], F32, tag="pr", name=rn("pr"))
    for t in range(16):
        for cc in range(2):
            nc.tensor.matmul(
                pr[:],
                mk8_sb[:, 2 * t : 2 * t + 2, 0:E],
                x8pair(t, slice(cc * 512, (cc + 1) * 512)),
                start=(t == 0 and cc == 0),
                stop=False,
                perf_mode=DR,
            )

    # --- T phase: tT[er, s] = (32*ldn).T @ x, fp8 DoubleRow ---
    for u in range(4):
        ps = [
            ptp.tile([P, 512], F32, tag="pt", name=rn(f"pt_{u}_{cc}"))
            for cc in range(2)
        ]
        for t in range(16):
            for cc in range(2):
                nc.tensor.matmul(
                    ps[cc][:],
                    ldcs[t // 8][:, 2 * (t % 8) : 2 * (t % 8) + 2, u * P : (u + 1) * P],
                    x8pair(t, slice(cc * 512, (cc + 1) * 512)),
                    start=(t == 0),
                    stop=(t == 15),
                    perf_mode=DR,
                )
        for cc in range(2):
            # rescale by 2^-10 (undoes the two host-side 32x prescales)
            nc.vector.tensor_scalar_mul(
                tT[:, u, cc * 512 : (cc + 1) * 512], ps[cc][:], ctsc[:]
            )

    # --- router projection, other sequence half (streamed fp8) ---
    for k in range(8):
        for jj in range(2):
            for cc in range(2):
                nc.tensor.matmul(
                    pr[:],
                    mk8_sb[:, 4 * k + 2 * jj : 4 * k + 2 * jj + 2, 0:E],
                    xo8q[k][:, 2 * jj : 2 * jj + 2, cc * 512 : (cc + 1) * 512],
                    start=False,
                    stop=(k == 7 and jj == 1 and cc == 1),
                    perf_mode=DR,
                )

    # --- softmax + coefficient placement ---
    scores_raw = rpool.tile([E, 1], F32, tag="scores_raw", name=rn("scores_raw"))
    nc.vector.reduce_sum(scores_raw[:], pr[:], axis=mybir.AxisListType.X)
    scores = rpool.tile([E, 1], F32, tag="scores", name=rn("scores"))
    nc.vector.tensor_scalar_mul(scores[:], scores_raw[:], csc8[:])
    exps = rpool.tile([E, 1], F32, tag="exps", name=rn("exps"))
    nc.scalar.activation(exps[:], scores[:], mybir.ActivationFunctionType.Exp)
    psum_s = pccp.tile([1, 1], F32, tag="pcc", name=rn("psum_s"))
    nc.tensor.matmul(psum_s[:], exps[:], ones8[:], start=True, stop=True)
    rinv = rpool.tile([1, 1], F32, tag="rinv", name=rn("rinv"))
    nc.vector.reciprocal(rinv[:], psum_s[:])
    rb_p = pccp.tile([P, 1], F32, tag="pcc", name=rn("rb_p"))
    nc.tensor.matmul(rb_p[:], ones_row[:], rinv[:], start=True, stop=True)
    rb = rpool.tile([P, 1], F32, tag="rb", name=rn("rb"))
    nc.vector.tensor_copy(rb[:], rb_p[:])
    # cc_un[pp, u] = exp(score[(u*128+pp)//64])
    cc_un = rpool.tile([P, 4], F32, tag="cc_un", name=rn("cc_un"))
    for u in range(4):
        pcc = pccp.tile([P, 1], F32, tag="pcc", name=rn(f"pcc_{u}"))
        nc.tensor.matmul(
            pcc[:],
            cind_sb[:, u * P : (u + 1) * P],
            exps[:],
            start=True,
            stop=True,
        )
        nc.vector.tensor_copy(cc_un[:, u : u + 1], pcc[:])
    coeff_f = rpool.tile([P, 4], F32, tag="coeff_f", name=rn("coeff_f"))
    nc.vector.tensor_scalar_mul(coeff_f[:], cc_un[:], rb[:])
    coeff = rpool.tile([P, 4], BF16, tag="coeff", name=rn("coeff"))
    nc.vector.tensor_copy(coeff[:], coeff_f[:])

    # --- coefficient-scaled lora_up tiles (bf16 in, fp8 out) ---
    lsc_tiles = [None] * NN

    def load_lsc(n):
        lraw = lrpool.tile([P, 4, 512], BF16, tag="lraw", name=rn(f"lraw_{n}"))
        nc.gpsimd.dma_start(
            lraw[:],
            lupb_ap[:, n * 512 : (n + 1) * 512].rearrange(
                "(u pp) o -> pp u o", pp=P
            ),
        )
        t = lpool.tile([P, 4, 512], FP8, tag="lsc", name=rn(f"lsc_{n}"))
        nc.vector.tensor_tensor(
            t[:],
            lraw[:],
            coeff[:, :, None].to_broadcast((P, 4, 512)),
            mybir.AluOpType.mult,
        )
        lsc_tiles[n] = t

    load_lsc(0)

    # --- main loop: out = x @ W + tT.T @ lsc, fused in PSUM ---
    for n in range(NN):
        if n + 1 < NN:
            wc_tiles[n + 1] = load_wc(n + 1)
        wc = wc_tiles.pop(n)
        lsc = lsc_tiles[n]
        for m in range(NM):
            sl = (
                slice(m * P, (m + 1) * P),
                slice(n * 512, (n + 1) * 512),
            )
            po = pop.tile([P, 512], F32, tag="po", name=rn(f"po_{n}_{m}"))
            for io in range(NIO):
                nc.tensor.matmul(
                    po[:],
                    xs(io, slice(m * P, (m + 1) * P)),
                    wc[io // 16][:, io % 16, :],
                    start=(io == 0),
                    stop=False,
                )
            for t in range(2):
                nc.tensor.matmul(
                    po[:],
                    tT[:, 2 * t : 2 * t + 2, m * P : (m + 1) * P],
                    lsc[:, 2 * t : 2 * t + 2, :],
                    start=False,
                    stop=(t == 1),
                    perf_mode=DR,
                )
            ost = opool.tile([P, 512], F32, tag="ost", name=rn(f"ost_{n}_{m}"))
            nc.vector.tensor_copy(ost[:], po[:])
            (nc.sync if m % 2 == 0 else nc.gpsimd).dma_start(out_ap[sl], ost[:])
            if m == 0 and n + 1 < NN:
                # prefetch next block's scaled lora_up after the first
                # eviction so a late lraw DMA cannot stall the DVE queue
                # ahead of this block's evictions
                load_lsc(n + 1)


def kernel(x, W_org, lora_down, lora_up, lora_keys, Wq):
    x = np.asarray(x, dtype=np.float32)
    W_org = np.asarray(W_org, dtype=np.float32)
    lora_down = np.asarray(lora_down, dtype=np.float32)
    lora_up = np.asarray(lora_up, dtype=np.float32)
    lora_keys = np.asarray(lora_keys, dtype=np.float32)
    Wq = np.asarray(Wq, dtype=np.float32)

    # Host-side constant folding / layout prep (transposes to K-major)
    wT = np.ascontiguousarray(W_org.T).astype(BF16_NP)               # [IN, OUT]
    ldn8 = (
        np.ascontiguousarray(lora_down.transpose(1, 0, 2).reshape(IN, ER))
        * LSHIFT
    ).astype(FP8_NP)                                                 # [IN, ER]
    lupb = (np.ascontiguousarray(lora_up.reshape(ER, OUT)) * LSHIFT).astype(
        BF16_NP
    )
    mk8 = np.zeros((IN, 2 * E), dtype=FP8_NP)                        # [IN, 16]
    mk8[:, :E] = (Wq.T @ lora_keys.T).astype(FP8_NP)
    cind = np.repeat(np.eye(E, dtype=np.float32), R, axis=1)         # [E, ER]
    xT = [np.ascontiguousarray(x[b].T) for b in range(B)]            # [IN, S] f32

    if "nc" not in _NC_CACHE:
        _NC_CACHE["nc"] = _build_nc()
    nc = _NC_CACHE["nc"]

    in_maps = []
    for c in range(8):
        b, h = c // 2, c % 2
        own = xT[b][:, h * SH : (h + 1) * SH]
        other = xT[b][:, (1 - h) * SH : (2 - h) * SH]
        in_maps.append(
            {
                "xh": own.astype(BF16_NP),
                "xh8": own.astype(FP8_NP),
                "xo8": other.astype(FP8_NP),
                "wT": wT,
                "ldn8": ldn8,
                "lupb": lupb,
                "mk8": mk8,
                "cind": cind,
            }
        )

    res = run_bass_kernel_spmd(nc, in_maps, core_ids=list(range(8)), trace=False)
    _NC_CACHE["last_result"] = res
    _NC_CACHE["last_in_maps"] = in_maps

    outp = np.empty((B, S, OUT), dtype=np.float32)
    for c in range(8):
        b, h = c // 2, c % 2
        outp[b, h * SH : (h + 1) * SH, :] = res.results[c]["out"]
    return outp


def _dispatch_fn(nc, in_maps):
    """Build a jitted single-NEFF dispatcher (mirrors run_bass_via_pjrt's
    multi-core path with device-resident inputs)."""
    import jax
    from jax.experimental.shard_map import shard_map
    from jax.sharding import Mesh, NamedSharding, PartitionSpec

    from concourse import bass2jax, mybir as _mybir

    bass2jax.install_neuronx_cc_hook()

    n_cores = len(in_maps)
    partition_name = nc.partition_id_tensor.name if nc.partition_id_tensor else None
    in_names, out_names, out_avals, zero_outs = [], [], [], []
    for alloc in nc.m.functions[0].allocations:
        if not isinstance(alloc, _mybir.MemoryLocationSet):
            continue
        name = alloc.memorylocations[0].name
        if alloc.kind == "ExternalInput":
            if name != partition_name:
                in_names.append(name)
        elif alloc.kind == "ExternalOutput":
            aval = jax.core.ShapedArray(
                tuple(alloc.tensor_shape), _mybir.dt.np(alloc.dtype)
            )
            out_avals.append(aval)
            out_names.append(name)
            zero_outs.append(np.zeros(aval.shape, aval.dtype))
    n_params = len(in_names)
    n_outs = len(out_avals)
    all_in_names = in_names + out_names
    if partition_name is not None:
        all_in_names = all_in_names + [partition_name]

    def _body(*args):
        operands = list(args)
        if partition_name is not None:
            operands.append(bass2jax.partition_id_tensor())
        outs = bass2jax._bass_exec_p.bind(
            *operands,
            out_avals=tuple(out_avals),
            in_names=tuple(all_in_names),
            out_names=tuple(out_names),
            lowering_input_output_aliases=(),
            sim_require_finite=True,
            sim_require_nnan=True,
            nc=nc,
        )
        return tuple(outs)

    _body.__name__ = "_body"

    devices = jax.devices()[:n_cores]
    mesh = Mesh(np.asarray(devices), ("core",))
    spec = PartitionSpec("core")
    sharding = NamedSharding(mesh, spec)
    donate = tuple(range(n_params, n_params + n_outs))
    fn = jax.jit(
        shard_map(
            _body,
            mesh=mesh,
            in_specs=(spec,) * (n_params + n_outs),
            out_specs=(spec,) * n_outs,
            check_rep=False,
        ),
        donate_argnums=donate,
        keep_unused=True,
    )

    concat_in = [
        np.concatenate([np.asarray(in_maps[c][nm]) for c in range(n_cores)], axis=0)
        for nm in in_names
    ]
    concat_zero = [
        np.zeros((n_cores * z.shape[0], *z.shape[1:]), z.dtype) for z in zero_outs
    ]
    dev_in = [jax.device_put(a, sharding) for a in concat_in]
    for a in dev_in:
        a.block_until_ready()
    return fn, dev_in, concat_zero, sharding


def _time_dispatch(nc, in_maps, iters):
    import time
    import jax

    fn, dev_in, concat_zero, sharding = _dispatch_fn(nc, in_maps)
    # outputs of call k are recycled as the donated out-operands of call
    # k+1 (same shape/dtype/sharding), so nothing re-ships between trials
    outs = [jax.device_put(z, sharding) for z in concat_zero]
    for z in outs:
        z.block_until_ready()
    times = []
    for _ in range(iters + 1):
        t0 = time.perf_counter()
        outs = fn(*dev_in, *outs)
        for o in outs:
            o.block_until_ready()
        times.append(time.perf_counter() - t0)
    return times[1:]


def benchmark_chained(iters=12, reps_hi=16):
    """Estimate per-execution device time as
    (min t(NEFF with reps_hi bodies) - min t(NEFF with 1 body)) / (reps_hi-1).
    The repeated-body NEFF serializes reps on-device, so the delta is pure
    device time; the axon relay dispatch noise cancels in the min."""
    if "nc" not in _NC_CACHE:
        _NC_CACHE["nc"] = _build_nc()
    key = f"nc_reps{reps_hi}"
    if key not in _NC_CACHE:
        _NC_CACHE[key] = _build_nc(reps=reps_hi)
    in_maps = _NC_CACHE["last_in_maps"]
    t1 = _time_dispatch(_NC_CACHE["nc"], in_maps, iters)
    tN = _time_dispatch(_NC_CACHE[key], in_maps, iters)
    per_exec = (min(tN) - min(t1)) / (reps_hi - 1)
    return per_exec, {1: t1, reps_hi: tN}


def benchmark(iters: int = 8, nc=None):
    """Single-dispatch wall times (relay noise included)."""
    if nc is None:
        nc = _NC_CACHE["nc"]
    return _time_dispatch(nc, _NC_CACHE["last_in_maps"], iters)


# revision 13
# speedup vs baseline: 43.6664x; 1.0002x over previous
"""AttLoRA MoE-routing kernel for 8 Trainium2 NeuronCores.

Reference computation (per problem nn_AttLoRAModule_85839216378078):
    base  = x @ W_org.T                                    [B,S,OUT]
    q     = x.mean(axis=1) @ Wq.T                          [B,K]
    coef  = softmax(q @ lora_keys.T / sqrt(K))             [B,E]
    h     = x @ lora_down[e]                               [B,S,E,R]
    delta = sum_e coef[b,e] * (h[...,e,:] @ lora_up[e])    [B,S,OUT]
    out   = base + delta * SCALE        (SCALE = 1.0 here)

Sharding: 8 cores = 4 batches x 2 sequence-halves.  Core c handles batch
c//2, tokens [(c%2)*1024, (c%2+1)*1024), all OUT columns.  The sequence
split (instead of an OUT split) halves the per-core x@lora_down work.

Device strategy (per core):
  - base GEMM in bf16 (1 PE cycle/row, = f32r rate, half the SBUF/DMA):
    x own-half [4096,1024] bf16 SBUF-resident; 64 PSUM groups of
    32 base matmuls + 2 fp8-DoubleRow delta matmuls; single eviction.
  - LoRA T-phase (x @ lora_down) and delta (tT.T @ lsc) in fp8e4m3 with
    perf_mode=DoubleRow (2 contraction rows/cycle): 131k -> 33k PE
    cycles each.  Host prescales lora_down/lora_up by 32 to dodge fp8
    subnormals; the tT eviction rescales by 2^-10.
  - Router entirely on PE in fp8 DoubleRow: pr[e,j] accumulates
    x[:,s] @ (Wq.T@keys.T) over BOTH sequence halves (other half
    streamed as fp8), then reduce + softmax on one partition and
    coefficients broadcast via indicator (cind) matmuls.  Coefficients
    are folded into bf16 lora_up tiles -> fp8 lsc.
  - PE work/core ~= 1.12M cycles ~= 468 us at warm 2.4 GHz.

The `reps` build parameter emits the whole computation N times in one
NEFF (tile tags shared across reps so SBUF slots rotate) — used only for
benchmarking: per-exec time = (t(reps_hi) - t(1)) / (reps_hi - 1), which
cancels the tens-of-ms axon relay dispatch noise.
"""

import math
import os

import numpy as np
import ml_dtypes

import concourse.bacc as bacc
import concourse.mybir as mybir
import concourse.tile as tile
from concourse.bass_utils import run_bass_kernel_spmd

# Problem shapes (hardcoded per contest contract)
B, S, IN, OUT = 4, 2048, 4096, 4096
E, R, K = 8, 64, 128
ER = E * R            # 512
SH = S // 2           # 1024 tokens per core
P = 128
NIO = IN // P         # 32 contraction subtiles
NN = OUT // 512       # 8 output column blocks
NM = SH // P          # 8 token row blocks
SCALE = 1.0           # (alpha/lora_dim) * multiplier
LSHIFT = 32.0         # host prescale of lora_down / lora_up (fp8 range)
TSCALE = 1.0 / (LSHIFT * LSHIFT)  # folded into the tT eviction

F32 = mybir.dt.float32
BF16 = mybir.dt.bfloat16
FP8 = mybir.dt.float8e4
BF16_NP = ml_dtypes.bfloat16
FP8_NP = ml_dtypes.float8_e4m3

_NC_CACHE = {}


def _build_nc(reps: int = 1):
    nc = bacc.Bacc("TRN2", target_bir_lowering=False, debug=False)

    xh = nc.dram_tensor("xh", [IN, SH], BF16, kind="ExternalInput")
    xh8 = nc.dram_tensor("xh8", [IN, SH], FP8, kind="ExternalInput")
    xo8 = nc.dram_tensor("xo8", [IN, SH], FP8, kind="ExternalInput")
    wT = nc.dram_tensor("wT", [IN, OUT], BF16, kind="ExternalInput")
    ldn8 = nc.dram_tensor("ldn8", [IN, ER], FP8, kind="ExternalInput")
    lupb = nc.dram_tensor("lupb", [ER, OUT], BF16, kind="ExternalInput")
    mk8 = nc.dram_tensor("mk8", [IN, 2 * E], FP8, kind="ExternalInput")
    cind = nc.dram_tensor("cind", [E, ER], F32, kind="ExternalInput")
    out = nc.dram_tensor("out", [SH, OUT], F32, kind="ExternalOutput")

    xh_ap, xh8_ap, xo8_ap, wT_ap, ldn8_ap, lupb_ap, mk8_ap, cind_ap, out_ap = (
        t.ap() for t in (xh, xh8, xo8, wT, ldn8, lupb, mk8, cind, out)
    )

    DR = mybir.MatmulPerfMode.DoubleRow
    trace_sim = os.environ.get("KERNEL_SIM_TRACE", "0") == "1"
    with tile.TileContext(nc, trace_sim=trace_sim) as tc:
        with (
            tc.tile_pool(name="xpool", bufs=1) as xpool,
            tc.tile_pool(name="x8pool", bufs=1) as x8pool,
            tc.tile_pool(name="xopool", bufs=2) as xopool,
            tc.tile_pool(name="spool", bufs=4) as spool,
            tc.tile_pool(name="tpool", bufs=1) as tpool,
            tc.tile_pool(name="lrpool", bufs=2) as lrpool,
            tc.tile_pool(name="lpool", bufs=2) as lpool,
            tc.tile_pool(name="opool", bufs=2) as opool,
            tc.tile_pool(name="rpool", bufs=1) as rpool,
            tc.tile_pool(name="ptp", bufs=2, space="PSUM") as ptp,
            tc.tile_pool(name="pop", bufs=4, space="PSUM") as pop,
            tc.tile_pool(name="prp", bufs=1, space="PSUM") as prp,
            tc.tile_pool(name="pccp", bufs=1, space="PSUM") as pccp,
        ):
            for rep in range(reps):
                _emit_once(
                    nc, rep,
                    xpool, x8pool, xopool, spool, tpool, lrpool, lpool,
                    opool, rpool, ptp, pop, prp, pccp,
                    xh_ap, xh8_ap, xo8_ap, wT_ap, ldn8_ap, lupb_ap, mk8_ap,
                    cind_ap, out_ap, DR,
                )

    nc.compile()
    return nc


def _emit_once(
    nc, rep,
    xpool, x8pool, xopool, spool, tpool, lrpool, lpool,
    opool, rpool, ptp, pop, prp, pccp,
    xh_ap, xh8_ap, xo8_ap, wT_ap, ldn8_ap, lupb_ap, mk8_ap, cind_ap, out_ap,
    DR,
):
    rn = lambda base: f"r{rep}_{base}"

    # --- persistent small tiles ---
    # E padded to 16 so the DoubleRow pair stride is 16B-aligned
    # (walrus s3_lw_dual_fp8_restrictions)
    mk8_sb = rpool.tile([P, NIO, 2 * E], FP8, tag="mk8_sb", name=rn("mk8_sb"))
    nc.sync.dma_start(mk8_sb[:], mk8_ap.rearrange("(io pp) e -> pp io e", pp=P))
    cind_sb = rpool.tile([E, ER], F32, tag="cind_sb", name=rn("cind_sb"))
    nc.scalar.dma_start(cind_sb[:], cind_ap)
    ones8 = rpool.tile([E, 1], F32, tag="ones8", name=rn("ones8"))
    nc.any.memset(ones8[:], 1.0)
    ones_row = rpool.tile([1, P], F32, tag="ones_row", name=rn("ones_row"))
    nc.any.memset(ones_row[:], 1.0)
    csc8 = rpool.tile([E, 1], F32, tag="csc8", name=rn("csc8"))
    nc.any.memset(csc8[:], 1.0 / (S * math.sqrt(K)))
    ctsc = rpool.tile([P, 1], F32, tag="ctsc", name=rn("ctsc"))
    nc.any.memset(ctsc[:], TSCALE)

    tT = tpool.tile([P, 4, SH], FP8, tag="tT", name=rn("tT"))

    pwarm = pccp.tile([P, P], F32, tag="pcc", name=rn("pwarm"))
    for w in range(7):
        nc.tensor.matmul(
            pwarm[:], ones_row[:], ones_row[:], start=(w == 0), stop=(w == 6)
        )

    # --- input streams (balanced across the two DMA paths) ---
    xq8 = []
    for k in range(16):
        t = x8pool.tile([P, 2, SH], FP8, tag=f"xh8_{k}", name=rn(f"xh8_{k}"))
        eng = nc.sync if k % 2 == 0 else nc.gpsimd
        eng.dma_start(
            t[:],
            xh8_ap[k * 256 : (k + 1) * 256, :].rearrange(
                "(io pp) s -> pp io s", pp=P
            ),
        )
        xq8.append(t)

    ldcs = []
    for j in range(2):
        ldc = spool.tile([P, 16, ER], FP8, tag="stream", name=rn(f"ldc{j}"))
        nc.scalar.dma_start(
            ldc[:],
            ldn8_ap[j * 2048 : (j + 1) * 2048, :].rearrange(
                "(io pp) e -> pp io e", pp=P
            ),
        )
        ldcs.append(ldc)

    xo8q = []
    for k in range(8):
        t = xopool.tile([P, 4, SH], FP8, tag="xo8", name=rn(f"xo8_{k}"))
        eng = nc.scalar
        eng.dma_start(
            t[:],
            xo8_ap[k * 512 : (k + 1) * 512, :].rearrange(
                "(io pp) s -> pp io s", pp=P
            ),
        )
        xo8q.append(t)

    def load_wc(n):
        wcs = []
        for j in range(2):
            wcj = spool.tile([P, 16, 512], BF16, tag="stream", name=rn(f"wc_{n}_{j}"))
            (nc.sync if j == 0 else nc.gpsimd).dma_start(
                wcj[:],
                wT_ap[
                    j * 2048 : (j + 1) * 2048, n * 512 : (n + 1) * 512
                ].rearrange("(io pp) o -> pp io o", pp=P),
            )
            wcs.append(wcj)
        return wcs

    wc_tiles = {0: load_wc(0)}

    xq = []
    for k in range(8):
        t = xpool.tile([P, 4, SH], BF16, tag=f"xh{k}", name=rn(f"xh{k}"))
        eng = nc.sync if k % 2 == 0 else nc.gpsimd
        eng.dma_start(
            t[:],
            xh_ap[k * 512 : (k + 1) * 512, :].rearrange(
                "(io pp) s -> pp io s", pp=P
            ),
        )
        xq.append(t)

    def xs(io, fslice):
        return xq[io // 4][:, io % 4, fslice]

    # fp8 x pair slice for DoubleRow: tile t holds exactly io pair (2t, 2t+1)
    def x8pair(t, fslice):
        return xq8[t][:, :, fslice]

    # --- router projection on PE, fp8 DoubleRow, both halves, one group ---
    pr = prp.tile([E, 512], F32, tag="pr", name=rn("pr"))
    for t in range(16):
        for cc in range(2):
            nc.tensor.matmul(
                pr[:],
                mk8_sb[:, 2 * t : 2 * t + 2, 0:E],
                x8pair(t, slice(cc * 512, (cc + 1) * 512)),
                start=(t == 0 and cc == 0),
                stop=False,
                perf_mode=DR,
            )

    # --- T phase: tT[er, s] = (32*ldn).T @ x, fp8 DoubleRow ---
    for u in range(4):
        ps = [
            ptp.tile([P, 512], F32, tag="pt", name=rn(f"pt_{u}_{cc}"))
            for cc in range(2)
        ]
        for t in range(16):
            for cc in range(2):
                nc.tensor.matmul(
                    ps[cc][:],
                    ldcs[t // 8][:, 2 * (t % 8) : 2 * (t % 8) + 2, u * P : (u + 1) * P],
                    x8pair(t, slice(cc * 512, (cc + 1) * 512)),
                    start=(t == 0),
                    stop=(t == 15),
                    perf_mode=DR,
                )
        for cc in range(2):
            # rescale by 2^-10 (undoes the two host-side 32x prescales)
            nc.vector.tensor_scalar_mul(
                tT[:, u, cc * 512 : (cc + 1) * 512], ps[cc][:], ctsc[:]
            )

    # --- router projection, other sequence half (streamed fp8) ---
    for k in range(8):
        for jj in range(2):
            for cc in range(2):
                nc.tensor.matmul(
                    pr[:],
                    mk8_sb[:, 4 * k + 2 * jj : 4 * k + 2 * jj + 2, 0:E],
                    xo8q[k][:, 2 * jj : 2 * jj + 2, cc * 512 : (cc + 1) * 512],
                    start=False,
                    stop=(k == 7 and jj == 1 and cc == 1),
                    perf_mode=DR,
                )

    # --- softmax + coefficient placement ---
    scores_raw = rpool.tile([E, 1], F32, tag="scores_raw", name=rn("scores_raw"))
    nc.vector.reduce_sum(scores_raw[:], pr[:], axis=mybir.AxisListType.X)
    scores = rpool.tile([E, 1], F32, tag="scores", name=rn("scores"))
    nc.vector.tensor_scalar_mul(scores[:], scores_raw[:], csc8[:])
    exps = rpool.tile([E, 1], F32, tag="exps", name=rn("exps"))
    nc.scalar.activation(exps[:], scores[:], mybir.ActivationFunctionType.Exp)
    psum_s = pccp.tile([1, 1], F32, tag="pcc", name=rn("psum_s"))
    nc.tensor.matmul(psum_s[:], exps[:], ones8[:], start=True, stop=True)
    rinv = rpool.tile([1, 1], F32, tag="rinv", name=rn("rinv"))
    nc.vector.reciprocal(rinv[:], psum_s[:])
    rb_p = pccp.tile([P, 1], F32, tag="pcc", name=rn("rb_p"))
    nc.tensor.matmul(rb_p[:], ones_row[:], rinv[:], start=True, stop=True)
    rb = rpool.tile([P, 1], F32, tag="rb", name=rn("rb"))
    nc.vector.tensor_copy(rb[:], rb_p[:])
    # cc_un[pp, u] = exp(score[(u*128+pp)//64])
    cc_un = rpool.tile([P, 4], F32, tag="cc_un", name=rn("cc_un"))
    for u in range(4):
        pcc = pccp.tile([P, 1], F32, tag="pcc", name=rn(f"pcc_{u}"))
        nc.tensor.matmul(
            pcc[:],
            cind_sb[:, u * P : (u + 1) * P],
            exps[:],
            start=True,
            stop=True,
        )
        nc.vector.tensor_copy(cc_un[:, u : u + 1], pcc[:])
    coeff_f = rpool.tile([P, 4], F32, tag="coeff_f", name=rn("coeff_f"))
    nc.vector.tensor_scalar_mul(coeff_f[:], cc_un[:], rb[:])
    coeff = rpool.tile([P, 4], BF16, tag="coeff", name=rn("coeff"))
    nc.vector.tensor_copy(coeff[:], coeff_f[:])

    # --- coefficient-scaled lora_up tiles (bf16 in, fp8 out) ---
    lsc_tiles = [None] * NN

    def load_lsc(n):
        lraw = lrpool.tile([P, 4, 512], BF16, tag="lraw", name=rn(f"lraw_{n}"))
        nc.gpsimd.dma_start(
            lraw[:],
            lupb_ap[:, n * 512 : (n + 1) * 512].rearrange(
                "(u pp) o -> pp u o", pp=P
            ),
        )
        t = lpool.tile([P, 4, 512], FP8, tag="lsc", name=rn(f"lsc_{n}"))
        nc.vector.tensor_tensor(
            t[:],
            lraw[:],
            coeff[:, :, None].to_broadcast((P, 4, 512)),
            mybir.AluOpType.mult,
        )
        lsc_tiles[n] = t

    load_lsc(0)

    # --- main loop: out = x @ W + tT.T @ lsc, fused in PSUM ---
    for n in range(NN):
        if n + 1 < NN:
            wc_tiles[n + 1] = load_wc(n + 1)
        wc = wc_tiles.pop(n)
        lsc = lsc_tiles[n]
        for m in range(NM):
            sl = (
                slice(m * P, (m + 1) * P),
                slice(n * 512, (n + 1) * 512),
            )
            po = pop.tile([P, 512], F32, tag="po", name=rn(f"po_{n}_{m}"))
            for io in range(NIO):
                nc.tensor.matmul(
                    po[:],
                    xs(io, slice(m * P, (m + 1) * P)),
                    wc[io // 16][:, io % 16, :],
                    start=(io == 0),
                    stop=False,
                )
            for t in range(2):
                nc.tensor.matmul(
                    po[:],
                    tT[:, 2 * t : 2 * t + 2, m * P : (m + 1) * P],
                    lsc[:, 2 * t : 2 * t + 2, :],
                    start=False,
                    stop=(t == 1),
                    perf_mode=DR,
                )
            ost = opool.tile([P, 512], F32, tag="ost", name=rn(f"ost_{n}_{m}"))
            if n == NN - 1 and m == NM - 1:
                for hh in range(2):
                    cs = slice(hh * 256, (hh + 1) * 256)
                    oc = slice(n * 512 + hh * 256, n * 512 + (hh + 1) * 256)
                    nc.vector.tensor_copy(ost[:, cs], po[:, cs])
                    (nc.sync if hh == 0 else nc.gpsimd).dma_start(
                        out_ap[sl[0], oc], ost[:, cs]
                    )
            else:
                nc.vector.tensor_copy(ost[:], po[:])
                (nc.sync if m % 2 == 0 else nc.gpsimd).dma_start(
                    out_ap[sl], ost[:]
                )
            if m == 0 and n + 1 < NN:
                # prefetch next block's scaled lora_up after the first
                # eviction so a late lraw DMA cannot stall the DVE queue
                # ahead of this block's evictions
                load_lsc(n + 1)


def kernel(x, W_org, lora_down, lora_up, lora_keys, Wq):
    x = np.asarray(x, dtype=np.float32)
    W_org = np.asarray(W_org, dtype=np.float32)
    lora_down = np.asarray(lora_down, dtype=np.float32)
    lora_up = np.asarray(lora_up, dtype=np.float32)
    lora_keys = np.asarray(lora_keys, dtype=np.float32)
    Wq = np.asarray(Wq, dtype=np.float32)

    # Host-side constant folding / layout prep (transposes to K-major)
    wT = np.ascontiguousarray(W_org.T).astype(BF16_NP)               # [IN, OUT]
    ldn8 = (
        np.ascontiguousarray(lora_down.transpose(1, 0, 2).reshape(IN, ER))
        * LSHIFT
    ).astype(FP8_NP)                                                 # [IN, ER]
    lupb = (np.ascontiguousarray(lora_up.reshape(ER, OUT)) * LSHIFT).astype(
        BF16_NP
    )
    mk8 = np.zeros((IN, 2 * E), dtype=FP8_NP)                        # [IN, 16]
    mk8[:, :E] = (Wq.T @ lora_keys.T).astype(FP8_NP)
    cind = np.repeat(np.eye(E, dtype=np.float32), R, axis=1)         # [E, ER]
    xT = [np.ascontiguousarray(x[b].T) for b in range(B)]            # [IN, S] f32

    if "nc" not in _NC_CACHE:
        _NC_CACHE["nc"] = _build_nc()
    nc = _NC_CACHE["nc"]

    in_maps = []
    for c in range(8):
        b, h = c // 2, c % 2
        own = xT[b][:, h * SH : (h + 1) * SH]
        other = xT[b][:, (1 - h) * SH : (2 - h) * SH]
        in_maps.append(
            {
                "xh": own.astype(BF16_NP),
                "xh8": own.astype(FP8_NP),
                "xo8": other.astype(FP8_NP),
                "wT": wT,
                "ldn8": ldn8,
                "lupb": lupb,
                "mk8": mk8,
                "cind": cind,
            }
        )

    res = run_bass_kernel_spmd(nc, in_maps, core_ids=list(range(8)), trace=False)
    _NC_CACHE["last_result"] = res
    _NC_CACHE["last_in_maps"] = in_maps

    outp = np.empty((B, S, OUT), dtype=np.float32)
    for c in range(8):
        b, h = c // 2, c % 2
        outp[b, h * SH : (h + 1) * SH, :] = res.results[c]["out"]
    return outp


def _dispatch_fn(nc, in_maps):
    """Build a jitted single-NEFF dispatcher (mirrors run_bass_via_pjrt's
    multi-core path with device-resident inputs)."""
    import jax
    from jax.experimental.shard_map import shard_map
    from jax.sharding import Mesh, NamedSharding, PartitionSpec

    from concourse import bass2jax, mybir as _mybir

    bass2jax.install_neuronx_cc_hook()

    n_cores = len(in_maps)
    partition_name = nc.partition_id_tensor.name if nc.partition_id_tensor else None
    in_names, out_names, out_avals, zero_outs = [], [], [], []
    for alloc in nc.m.functions[0].allocations:
        if not isinstance(alloc, _mybir.MemoryLocationSet):
            continue
        name = alloc.memorylocations[0].name
        if alloc.kind == "ExternalInput":
            if name != partition_name:
                in_names.append(name)
        elif alloc.kind == "ExternalOutput":
            aval = jax.core.ShapedArray(
                tuple(alloc.tensor_shape), _mybir.dt.np(alloc.dtype)
            )
            out_avals.append(aval)
            out_names.append(name)
            zero_outs.append(np.zeros(aval.shape, aval.dtype))
    n_params = len(in_names)
    n_outs = len(out_avals)
    all_in_names = in_names + out_names
    if partition_name is not None:
        all_in_names = all_in_names + [partition_name]

    def _body(*args):
        operands = list(args)
        if partition_name is not None:
            operands.append(bass2jax.partition_id_tensor())
        outs = bass2jax._bass_exec_p.bind(
            *operands,
            out_avals=tuple(out_avals),
            in_names=tuple(all_in_names),
            out_names=tuple(out_names),
            lowering_input_output_aliases=(),
            sim_require_finite=True,
            sim_require_nnan=True,
            nc=nc,
        )
        return tuple(outs)

    _body.__name__ = "_body"

    devices = jax.devices()[:n_cores]
    mesh = Mesh(np.asarray(devices), ("core",))
    spec = PartitionSpec("core")
    sharding = NamedSharding(mesh, spec)
    donate = tuple(range(n_params, n_params + n_outs))
    fn = jax.jit(
        shard_map(
            _body,
            mesh=mesh,
            in_specs=(spec,) * (n_params + n_outs),
            out_specs=(spec,) * n_outs,
            check_rep=False,
        ),
        donate_argnums=donate,
        keep_unused=True,
    )

    concat_in = [
        np.concatenate([np.asarray(in_maps[c][nm]) for c in range(n_cores)], axis=0)
        for nm in in_names
    ]
    concat_zero = [
        np.zeros((n_cores * z.shape[0], *z.shape[1:]), z.dtype) for z in zero_outs
    ]
    dev_in = [jax.device_put(a, sharding) for a in concat_in]
    for a in dev_in:
        a.block_until_ready()
    return fn, dev_in, concat_zero, sharding


def _time_dispatch(nc, in_maps, iters):
    import time
    import jax

    fn, dev_in, concat_zero, sharding = _dispatch_fn(nc, in_maps)
    # outputs of call k are recycled as the donated out-operands of call
    # k+1 (same shape/dtype/sharding), so nothing re-ships between trials
    outs = [jax.device_put(z, sharding) for z in concat_zero]
    for z in outs:
        z.block_until_ready()
    times = []
    for _ in range(iters + 1):
        t0 = time.perf_counter()
        outs = fn(*dev_in, *outs)
        for o in outs:
            o.block_until_ready()
        times.append(time.perf_counter() - t0)
    return times[1:]


def benchmark_chained(iters=12, reps_hi=16):
    """Estimate per-execution device time as
    (min t(NEFF with reps_hi bodies) - min t(NEFF with 1 body)) / (reps_hi-1).
    The repeated-body NEFF serializes reps on-device, so the delta is pure
    device time; the axon relay dispatch noise cancels in the min."""
    if "nc" not in _NC_CACHE:
        _NC_CACHE["nc"] = _build_nc()
    key = f"nc_reps{reps_hi}"
    if key not in _NC_CACHE:
        _NC_CACHE[key] = _build_nc(reps=reps_hi)
    in_maps = _NC_CACHE["last_in_maps"]
    t1 = _time_dispatch(_NC_CACHE["nc"], in_maps, iters)
    tN = _time_dispatch(_NC_CACHE[key], in_maps, iters)
    per_exec = (min(tN) - min(t1)) / (reps_hi - 1)
    return per_exec, {1: t1, reps_hi: tN}


def benchmark(iters: int = 8, nc=None):
    """Single-dispatch wall times (relay noise included)."""
    if nc is None:
        nc = _NC_CACHE["nc"]
    return _time_dispatch(nc, _NC_CACHE["last_in_maps"], iters)
